# revision 10
# baseline (speedup 1.0000x reference)
"""Trainium2 Bass kernel for nn_AccLSTMCell (v2).

Model (per time step t, per batch row):
    a   = concat(x_t, acc) @ W_in + b_in            (linear)
    h0,c0 = LSTMCell(a,  h0, c0; k0, rk0, bb0)
    h1,c1 = LSTMCell(h0, h1, c1; k1, rk1, bb1)
    res = h1 @ W_out + b_out
    acc = acc + res ;  output[t] = res

Key structure (v2):
  * Data parallel: batch 4096 -> 512 rows per NeuronCore (8 cores),
    each core splits its 512 rows into 2 chunks of 256 for pipelining.
  * Layout: features/units on SBUF partitions, batch on free dim.
  * W_in folded into k0 host-side and split into three z0 contributors:
      x-part   Wx  = W_in[:64]  @ k0   (K=64,  dep-free)
      acc-part Wa  = W_in[64:96]@ k0   (K=32,  reads fp16 acc shadow)
      res-part Wh  = W_out @ Wa        (K=128, reads h1 directly)
    z0(t+1) = x(t+1)@Wx + shadow(t-1)@Wa + h0(t)@rk0 + h1(t)@Wh.
    The Wh path keeps the per-step wout->cast sequence OFF the critical
    recurrence chain (the acc shadow cast has a full step of slack).
  * All four gates go through ONE sigmoid per cell: the g columns of
    every z-contributing weight are pre-scaled by 2 host-side, so
    sigmoid(2g) = (tanh(g)+1)/2 and the DVE fixes tg = 2*s_g - 1.
  * fp16 matmuls (fp32 PSUM), fp16 gates/h/c, fp32 acc in PSUM
    (accumulated for free by the wout matmul, start=False).
  * Kernel emits acc history; host recovers res[t] = acc[t]-acc[t-1].
  * Emission interleaves the two chunks at CELL granularity (chunk B
    lags chunk A by one cell) so each engine always has the other
    chunk's ready work behind the current op (hides cross-engine
    semaphore latency; keeps the PE dense so HAM stays at 2.4 GHz).
"""

import os

import numpy as np

import concourse.bass as bass
import concourse.bacc as bacc
import concourse.tile as tile
from concourse import mybir
from concourse.bass_utils import run_bass_kernel_spmd

# Problem constants (hardcoded; harness contract).
B_FULL, T_FULL, F_IN = 4096, 256, 64
N_OUT = 32
ANN = 128          # ann_in dense width
U = 128            # units of both LSTM cells
NCORES = 8
BL_FULL = B_FULL // NCORES   # 512 batch rows per core

FP16 = mybir.dt.float16
FP32 = mybir.dt.float32
AF = mybir.ActivationFunctionType
ALU = mybir.AluOpType

TBLK = int(os.environ.get("LSTM_TBLK", "16"))   # time steps per DMA block
NCH = 2                                          # batch chunks per core
ZBUFS = int(os.environ.get("LSTM_ZBUFS", "3"))
GBUFS = int(os.environ.get("LSTM_GBUFS", "3"))
TRACE = os.environ.get("LSTM_TRACE", "0") == "1"

PSUM_BANK_BYTES = 2048


def build_program(T=T_FULL, BL=BL_FULL, tblk=TBLK, nch=NCH):
    """Build the per-core Bass program. Returns (nc, meta)."""
    assert T % tblk == 0 and BL % nch == 0
    nblk = T // tblk
    ch = BL // nch
    assert nch == 2, "emission interleave assumes two chunks"

    nc = bacc.Bacc("TRN2", name="acclstm2")

    xd = nc.dram_tensor("x", [nblk, F_IN, tblk, BL], FP16, kind="ExternalInput")
    accd = nc.dram_tensor("acc_hist", [nblk, N_OUT, tblk, BL], FP32,
                          kind="ExternalOutput")
    # Weights: column order is PSUM-slot order [i, f, o, g] (host reorders;
    # g columns pre-scaled by 2 for the sigmoid-only gate trick).
    kxd = nc.dram_tensor("kx", [F_IN + N_OUT, 4 * U], FP16,
                         kind="ExternalInput")
    whd = nc.dram_tensor("wh", [U, 4 * U], FP16, kind="ExternalInput")
    rk0d = nc.dram_tensor("rk0", [U, 4 * U], FP16, kind="ExternalInput")
    k1d = nc.dram_tensor("k1", [U, 4 * U], FP16, kind="ExternalInput")
    rk1d = nc.dram_tensor("rk1", [U, 4 * U], FP16, kind="ExternalInput")
    woutd = nc.dram_tensor("wout", [U, N_OUT], FP16, kind="ExternalInput")

    slots_per_bank = max(1, PSUM_BANK_BYTES // (ch * 4))

    with tile.TileContext(nc) as tc:
        with (
            tc.tile_pool(name="wpool", bufs=1) as wpool,
            tc.tile_pool(name="state", bufs=1) as state,
            tc.tile_pool(name="xpool", bufs=2) as xpool,
            tc.tile_pool(name="opool", bufs=2) as opool,
            tc.tile_pool(name="gates", bufs=GBUFS) as gates,
            tc.tile_pool(name="zpool", bufs=ZBUFS, space="PSUM") as zpool,
            tc.tile_pool(name="apool", bufs=1, space="PSUM") as apool,
        ):
            # --- weights -> SBUF ---
            def wload(nm, dram, shape):
                t_ = wpool.tile(shape, FP16, name=nm, tag=nm)
                nc.sync.dma_start(out=t_, in_=dram[:, :])
                return t_
            kx = wload("kx", kxd, [F_IN + N_OUT, 4 * U])
            wh = wload("wh", whd, [U, 4 * U])
            rk0 = wload("rk0", rk0d, [U, 4 * U])
            k1 = wload("k1", k1d, [U, 4 * U])
            rk1 = wload("rk1", rk1d, [U, 4 * U])
            wout = wload("wout", woutd, [U, N_OUT])

            # --- persistent per-chunk state ---
            h0s, h1s, tgc0s, tgc1s = [], [], [], []
            for c in range(nch):
                h0 = state.tile([U, ch], FP16, name=f"h0_{c}", tag=f"h0_{c}")
                h1 = state.tile([U, ch], FP16, name=f"h1_{c}", tag=f"h1_{c}")
                # [tg | c] combined tile per cell: tg scratch, c persistent.
                tgc0 = state.tile([U, 2 * ch], FP16, name=f"tgc0_{c}",
                                  tag=f"tgc0_{c}")
                tgc1 = state.tile([U, 2 * ch], FP16, name=f"tgc1_{c}",
                                  tag=f"tgc1_{c}")
                for tl in (h0, h1, tgc0, tgc1):
                    nc.vector.memset(tl, 0.0)
                h0s.append(h0); h1s.append(h1)
                tgc0s.append(tgc0); tgc1s.append(tgc1)

            # fp32 acc accumulators live in PSUM, one bank per chunk,
            # updated by the wout matmul itself (start=False accumulate).
            accps = [apool.tile([N_OUT, ch], FP32, name=f"accps{c}",
                                tag=f"accps{c}") for c in range(nch)]

            # --- x input blocks / output history blocks ---
            xts = [None] * nblk
            ots = [None] * nblk

            def alloc_xblock(b):
                # xa blocks: x on partitions 0:64 (DMA); fp16 acc shadow on
                # partitions 64:96, written by the cast for step t+2 (so it
                # is never on the recurrence critical path).
                xts[b] = xpool.tile([F_IN + N_OUT, tblk, BL], FP16,
                                    name=f"xb{b}", tag="xblk")
                nc.sync.dma_start(out=xts[b][0:F_IN], in_=xd[b])

            alloc_xblock(0)
            # shadow for steps 0 and 1 is acc(-2) = acc(-1) = 0
            nc.vector.memset(xts[0][F_IN:, 0:2, :], 0.0)
            alloc_xblock(1)

            def mm_half(z, chain, w, rhs, is_first, is_last):
                """Emit the 4 gate-slot matmuls of one z contributor.
                Within each 2KB PSUM bank the first executed matmul must
                carry start=True and the last stop=True; `chain` pins the
                execution order inside each bank with same-engine deps."""
                for bank0 in range(0, 4, slots_per_bank):
                    bslots = list(range(bank0, min(bank0 + slots_per_bank, 4)))
                    bk = bank0 // slots_per_bank
                    for i, s in enumerate(bslots):
                        mm = nc.tensor.matmul(
                            z[:, s * ch:(s + 1) * ch],
                            w[:, s * U:(s + 1) * U],
                            rhs,
                            start=(is_first and i == 0),
                            stop=(is_last and i == len(bslots) - 1),
                        )
                        prev = chain.get(bk)
                        if prev is not None:
                            tile.add_dep_helper(
                                mm.ins, prev.ins, sync=False,
                                reason="psum bank group order")
                        chain[bk] = mm

            # Per-chunk in-flight z tiles / chains.
            z0t = [None] * nch    # z0 tile consumed by sig0 at step t
            z1t = [None] * nch
            z0n = [None] * nch    # z0 tile being assembled for step t+1
            z0n_chain = [None] * nch

            def start_z0(c, t):
                """Open z0 for step t: [x(t); acc(t-2)] @ kx + rk0(h0(t-1)).
                Wh(h1(t-1)) closes the group, adding the missing res(t-1)
                contribution: acc(t-1) = acc(t-2) + res(t-1)."""
                zt = zpool.tile([U, 4 * ch], FP32, name="z0", tag="z")
                chain = {}
                b, j = t // tblk, t % tblk
                lo, hi = c * ch, (c + 1) * ch
                mm_half(zt, chain, kx, xts[b][:, j, lo:hi], True, False)
                mm_half(zt, chain, rk0, h0s[c], False, t == 0)
                z0n[c] = zt
                z0n_chain[c] = chain

            def finish_z0(c):
                """Close z0(t+1) with the Wh(h1(t)) res-part."""
                mm_half(z0n[c], z0n_chain[c], wh, h1s[c], False, True)
                z0t[c] = z0n[c]
                z0n[c] = None

            # --- per-(chunk, cell) elementwise phases ---
            st = [{}, {}]

            def p_sig(c, cell):
                z = z0t[c] if cell == 0 else z1t[c]
                sio = gates.tile([U, 4 * ch], FP16, name=f"sio{cell}",
                                 tag="sio")
                nc.scalar.activation(sio, z, AF.Sigmoid)
                st[c][f"sio{cell}"] = sio

            def p_dve(c, cell):
                sio = st[c][f"sio{cell}"]
                tgc = (tgc0s if cell == 0 else tgc1s)[c]
                uv = gates.tile([U, 2 * ch], FP16, name="uv", tag="uv")
                # tg = 2*sigmoid(2g) - 1 = tanh(g)
                nc.vector.tensor_scalar(
                    tgc[:, 0:ch], sio[:, 3 * ch:4 * ch],
                    2.0, 1.0, ALU.mult, ALU.subtract)
                # [u|v] = [s_i|s_f] * [tg|c]
                nc.vector.tensor_tensor(
                    out=uv, in0=sio[:, 0:2 * ch], in1=tgc, op=ALU.mult)
                # c_new = u + v  (written into the c half of tgc)
                nc.vector.tensor_tensor(
                    out=tgc[:, ch:2 * ch], in0=uv[:, 0:ch],
                    in1=uv[:, ch:2 * ch], op=ALU.add)

            def p_tanh(c, cell):
                tgc = (tgc0s if cell == 0 else tgc1s)[c]
                tc_t = gates.tile([U, ch], FP16, name="tc", tag="tc")
                nc.scalar.activation(tc_t, tgc[:, ch:2 * ch], AF.Tanh)
                st[c][f"tc{cell}"] = tc_t

            def p_h(c, cell):
                sio = st[c][f"sio{cell}"]
                hstate = (h0s if cell == 0 else h1s)[c]
                nc.vector.tensor_tensor(
                    out=hstate, in0=sio[:, 2 * ch:3 * ch],
                    in1=st[c][f"tc{cell}"], op=ALU.mult)

            def p_pe0(c, t):
                """After h0(t): z1(t) = rk1(h1(t-1)) + k1(h0(t)); also open
                z0(t+1)."""
                zt = zpool.tile([U, 4 * ch], FP32, name="z1", tag="z")
                chain = {}
                mm_half(zt, chain, rk1, h1s[c], True, False)
                mm_half(zt, chain, k1, h0s[c], False, True)
                z1t[c] = zt
                if t + 1 < T:
                    start_z0(c, t + 1)

            def p_pe1(c, t):
                """After h1(t): close z0(t+1) with Wh; acc += res via wout."""
                if t + 1 < T:
                    finish_z0(c)
                nc.tensor.matmul(accps[c], wout, h1s[c],
                                 start=(t == 0), stop=True,
                                 skip_group_check=(t > 0))

            def p_tail(c, t):
                """acc snapshot -> fp32 history; acc -> fp16 shadow for the
                step-(t+2) kx matmuls (cast reads the SBUF history copy,
                which is cheaper than a second PSUM read)."""
                b, j = t // tblk, t % tblk
                lo, hi = c * ch, (c + 1) * ch
                nc.vector.tensor_copy(out=ots[b][:, j, lo:hi], in_=accps[c])
                if t + 2 < T:
                    t2 = t + 2
                    b2, j2 = t2 // tblk, t2 % tblk
                    nc.vector.tensor_copy(
                        out=xts[b2][F_IN:, j2, lo:hi],
                        in_=ots[b][:, j, lo:hi])

            # --- prologue: z0(0) for both chunks ---
            for c in range(nch):
                start_z0(c, 0)
                z0t[c] = z0n[c]
                z0n[c] = None

            A, B = 0, 1
            for t in range(T):
                b, j = t // tblk, t % tblk
                if j == 0:
                    if b + 2 < nblk and xts[b + 2] is None:
                        alloc_xblock(b + 2)
                    ots[b] = opool.tile([N_OUT, tblk, BL], FP32,
                                        name=f"ob{b}", tag="oblk")
                # slot 1-5: A cell0(t) with B cell1(t-1) trailing
                p_sig(A, 0)
                if t > 0:
                    p_sig(B, 1)
                p_dve(A, 0)
                if t > 0:
                    p_dve(B, 1)
                p_tanh(A, 0)
                if t > 0:
                    p_tanh(B, 1)
                p_h(A, 0)
                if t > 0:
                    p_h(B, 1)
                p_pe0(A, t)
                if t > 0:
                    p_pe1(B, t - 1)
                    p_tail(B, t - 1)
                    if j == 0:
                        # B's tail for the last row of block b-1 just ran.
                        nc.sync.dma_start(out=accd[b - 1], in_=ots[b - 1])
                # slot 6-10: A cell1(t) with B cell0(t)
                p_sig(A, 1)
                p_sig(B, 0)
                p_dve(A, 1)
                p_dve(B, 0)
                p_tanh(A, 1)
                p_tanh(B, 0)
                p_h(A, 1)
                p_h(B, 0)
                p_pe1(A, t)
                p_tail(A, t)
                p_pe0(B, t)

            # epilogue: B's cell1 of the last step + final DMA
            t = T - 1
            p_sig(B, 1)
            p_dve(B, 1)
            p_tanh(B, 1)
            p_h(B, 1)
            p_pe1(B, t)
            p_tail(B, t)
            nc.sync.dma_start(out=accd[nblk - 1], in_=ots[nblk - 1])

    nc.compile()
    meta = dict(T=T, BL=BL, tblk=tblk, nblk=nblk, nch=nch, ch=ch)
    return nc, meta


# Column reorder: reference gate order in z is [i, f, g, o]; PSUM slot
# order is [i, f, o, g] so sigmoid covers slots 0..2 contiguously and g
# sits in the last slot for the tg fix-up.
def _reorder_cols(w):
    u = w.shape[1] // 4
    return np.concatenate(
        [w[:, 0:u], w[:, u:2 * u], w[:, 3 * u:4 * u], w[:, 2 * u:3 * u]], axis=1)


def _prep_z_weight(w):
    """Reorder to [i,f,o,g] and scale the g columns by 2 (sigmoid trick)."""
    w = _reorder_cols(w)
    u = w.shape[1] // 4
    w = w.copy()
    w[:, 3 * u:] *= 2.0
    return w.astype(np.float16)


def prep_weights(W_in, b_in, k0, rk0, bb0, k1, rk1, bb1, W_out, b_out):
    assert np.allclose(b_in, 0) and np.allclose(bb0, 0) and np.allclose(bb1, 0), \
        "nonzero ann/lstm biases not supported by this kernel build"
    k0f = np.asarray(k0, dtype=np.float64)
    Wf = np.asarray(W_in, dtype=np.float64)
    kxf = (Wf @ k0f).astype(np.float32)                 # [96, 512]
    Wh = (np.asarray(W_out, np.float64) @ (Wf[F_IN:] @ k0f)).astype(np.float32)
    return {
        "kx": _prep_z_weight(kxf),
        "wh": _prep_z_weight(Wh),
        "rk0": _prep_z_weight(np.asarray(rk0, np.float32)),
        "k1": _prep_z_weight(np.asarray(k1, np.float32)),
        "rk1": _prep_z_weight(np.asarray(rk1, np.float32)),
        "wout": np.asarray(W_out).astype(np.float16),
    }


def prep_x_core(x_core, tblk):
    """[BL, T, F] fp32 -> [nblk, F, tblk, BL] fp16."""
    BL, T, F = x_core.shape
    nblk = T // tblk
    xt = np.ascontiguousarray(x_core.transpose(1, 2, 0))       # [T, F, BL]
    xt = xt.reshape(nblk, tblk, F, BL).transpose(0, 2, 1, 3)   # [nblk,F,tblk,BL]
    return np.ascontiguousarray(xt).astype(np.float16)


def post_acc_core(acc_hist, b_out):
    """[nblk, 32, tblk, BL] fp32 acc history -> [BL, T, 32] res."""
    nblk, n_out, tblk, BL = acc_hist.shape
    acc = acc_hist.transpose(0, 2, 3, 1).reshape(nblk * tblk, BL, n_out)
    res = np.empty_like(acc)
    res[0] = acc[0]
    np.subtract(acc[1:], acc[:-1], out=res[1:])
    out = res.transpose(1, 0, 2) + b_out.astype(np.float32)
    return np.ascontiguousarray(out.astype(np.float32))


def kernel(inputs, W_in, b_in, k0, rk0, bb0, k1, rk1, bb1, W_out, b_out):
    inputs = np.asarray(inputs, dtype=np.float32)
    W_in, b_in, k0, rk0, bb0, k1, rk1, bb1, W_out, b_out = (
        np.asarray(a, dtype=np.float32)
        for a in (W_in, b_in, k0, rk0, bb0, k1, rk1, bb1, W_out, b_out))
    weights = prep_weights(W_in, b_in, k0, rk0, bb0, k1, rk1, bb1, W_out, b_out)

    nc, meta = build_program()
    in_maps = []
    for r in range(NCORES):
        x_core = inputs[r * BL_FULL:(r + 1) * BL_FULL]
        m = dict(weights)
        m["x"] = prep_x_core(x_core, meta["tblk"])
        in_maps.append(m)

    ret = run_bass_kernel_spmd(nc, in_maps, core_ids=list(range(NCORES)),
                               trace=TRACE)
    if TRACE:
        print("exec_time_ns:", ret.exec_time_ns,
              "mean:", ret.mean_exec_time_ns)
        if ret.instructions_and_trace is not None:
            print("trace:", ret.instructions_and_trace[1])
        kernel.last_results = ret

    out = np.empty((B_FULL, T_FULL, N_OUT), dtype=np.float32)
    for r in range(NCORES):
        out[r * BL_FULL:(r + 1) * BL_FULL] = post_acc_core(
            ret.results[r]["acc_hist"], np.asarray(b_out))
    return out


# revision 19
# speedup vs baseline: 1.0148x; 1.0148x over previous
"""Trainium2 Bass kernel for nn_AccLSTMCell (v2).

Model (per time step t, per batch row):
    a   = concat(x_t, acc) @ W_in + b_in            (linear)
    h0,c0 = LSTMCell(a,  h0, c0; k0, rk0, bb0)
    h1,c1 = LSTMCell(h0, h1, c1; k1, rk1, bb1)
    res = h1 @ W_out + b_out
    acc = acc + res ;  output[t] = res

Key structure (v2):
  * Data parallel: batch 4096 -> 512 rows per NeuronCore (8 cores),
    each core splits its 512 rows into 2 chunks of 256 for pipelining.
  * Layout: features/units on SBUF partitions, batch on free dim.
  * W_in folded into k0 host-side and split into three z0 contributors:
      x-part   Wx  = W_in[:64]  @ k0   (K=64,  dep-free)
      acc-part Wa  = W_in[64:96]@ k0   (K=32,  reads fp16 acc shadow)
      res-part Wh  = W_out @ Wa        (K=128, reads h1 directly)
    z0(t+1) = x(t+1)@Wx + shadow(t-1)@Wa + h0(t)@rk0 + h1(t)@Wh.
    The Wh path keeps the per-step wout->cast sequence OFF the critical
    recurrence chain (the acc shadow cast has a full step of slack).
  * All four gates go through ONE sigmoid per cell: the g columns of
    every z-contributing weight are pre-scaled by 2 host-side, so
    sigmoid(2g) = (tanh(g)+1)/2 and the DVE fixes tg = 2*s_g - 1.
  * fp16 matmuls (fp32 PSUM), fp16 gates/h/c, fp32 acc in PSUM
    (accumulated for free by the wout matmul, start=False).
  * Kernel emits acc history; host recovers res[t] = acc[t]-acc[t-1].
  * Emission interleaves the two chunks at CELL granularity (chunk B
    lags chunk A by one cell) so each engine always has the other
    chunk's ready work behind the current op (hides cross-engine
    semaphore latency; keeps the PE dense so HAM stays at 2.4 GHz).
"""

import os

import numpy as np

import concourse.bass as bass
import concourse.bacc as bacc
import concourse.tile as tile
from concourse import mybir
from concourse.bass_utils import run_bass_kernel_spmd

# Problem constants (hardcoded; harness contract).
B_FULL, T_FULL, F_IN = 4096, 256, 64
N_OUT = 32
ANN = 128          # ann_in dense width
U = 128            # units of both LSTM cells
NCORES = 8
BL_FULL = B_FULL // NCORES   # 512 batch rows per core

FP16 = mybir.dt.float16
FP32 = mybir.dt.float32
AF = mybir.ActivationFunctionType
ALU = mybir.AluOpType

TBLK = int(os.environ.get("LSTM_TBLK", "16"))   # time steps per DMA block
NCH = 2                                          # batch chunks per core
ZBUFS = int(os.environ.get("LSTM_ZBUFS", "3"))
GBUFS = int(os.environ.get("LSTM_GBUFS", "3"))
TRACE = os.environ.get("LSTM_TRACE", "0") == "1"

PSUM_BANK_BYTES = 2048


def build_program(T=T_FULL, BL=BL_FULL, tblk=TBLK, nch=NCH):
    """Build the per-core Bass program. Returns (nc, meta)."""
    assert T % tblk == 0 and BL % nch == 0
    nblk = T // tblk
    ch = BL // nch
    assert nch == 2, "emission interleave assumes two chunks"

    nc = bacc.Bacc("TRN2", name="acclstm2")

    xd = nc.dram_tensor("x", [nblk, F_IN, tblk, BL], FP16, kind="ExternalInput")
    accd = nc.dram_tensor("acc_hist", [nblk, N_OUT, tblk, BL], FP32,
                          kind="ExternalOutput")
    # Weights: column order is PSUM-slot order [i, f, o, g] (host reorders;
    # g columns pre-scaled by 2 for the sigmoid-only gate trick).
    kxXd = nc.dram_tensor("kxX", [F_IN, 4 * U], FP16, kind="ExternalInput")
    kxAd = nc.dram_tensor("kxA", [N_OUT, 4 * U], FP16, kind="ExternalInput")
    whd = nc.dram_tensor("wh", [U, 4 * U], FP16, kind="ExternalInput")
    rk0d = nc.dram_tensor("rk0", [U, 4 * U], FP16, kind="ExternalInput")
    k1d = nc.dram_tensor("k1", [U, 4 * U], FP16, kind="ExternalInput")
    rk1d = nc.dram_tensor("rk1", [U, 4 * U], FP16, kind="ExternalInput")
    woutd = nc.dram_tensor("wout", [U, N_OUT], FP16, kind="ExternalInput")

    slots_per_bank = max(1, PSUM_BANK_BYTES // (ch * 4))

    with tile.TileContext(nc) as tc:
        with (
            tc.tile_pool(name="wpool", bufs=1) as wpool,
            tc.tile_pool(name="state", bufs=1) as state,
            tc.tile_pool(name="xpool", bufs=2) as xpool,
            tc.tile_pool(name="opool", bufs=2) as opool,
            tc.tile_pool(name="gates", bufs=GBUFS) as gates,
            tc.tile_pool(name="zpool", bufs=ZBUFS, space="PSUM") as zpool,
            tc.tile_pool(name="apool", bufs=1, space="PSUM") as apool,
        ):
            # --- weights -> SBUF ---
            def wload(nm, dram, shape):
                t_ = wpool.tile(shape, FP16, name=nm, tag=nm)
                nc.sync.dma_start(out=t_, in_=dram[:, :])
                return t_
            kxX = wload("kxX", kxXd, [F_IN, 4 * U])
            kxA = wload("kxA", kxAd, [N_OUT, 4 * U])
            wh = wload("wh", whd, [U, 4 * U])
            rk0 = wload("rk0", rk0d, [U, 4 * U])
            k1 = wload("k1", k1d, [U, 4 * U])
            rk1 = wload("rk1", rk1d, [U, 4 * U])
            wout = wload("wout", woutd, [U, N_OUT])

            # --- persistent per-chunk state ---
            h0s, h1s, tgc0s, tgc1s, shads = [], [], [], [], []
            for c in range(nch):
                h0 = state.tile([U, ch], FP16, name=f"h0_{c}", tag=f"h0_{c}")
                h1 = state.tile([U, ch], FP16, name=f"h1_{c}", tag=f"h1_{c}")
                # [tg | c] combined tile per cell: tg scratch, c persistent.
                tgc0 = state.tile([U, 2 * ch], FP16, name=f"tgc0_{c}",
                                  tag=f"tgc0_{c}")
                tgc1 = state.tile([U, 2 * ch], FP16, name=f"tgc1_{c}",
                                  tag=f"tgc1_{c}")
                sh = [state.tile([N_OUT, ch], FP16, name=f"sh{k}_{c}",
                                 tag=f"sh{k}_{c}") for k in range(2)]
                for tl in (h0, h1, tgc0, tgc1, sh[0], sh[1]):
                    nc.vector.memset(tl, 0.0)
                h0s.append(h0); h1s.append(h1)
                tgc0s.append(tgc0); tgc1s.append(tgc1); shads.append(sh)

            # fp32 acc accumulators live in PSUM, one bank per chunk,
            # updated by the wout matmul itself (start=False accumulate).
            accps = [apool.tile([N_OUT, ch], FP32, name=f"accps{c}",
                                tag=f"accps{c}") for c in range(nch)]

            # --- x input blocks / output history blocks ---
            xts = [None] * nblk
            ots = [None] * nblk

            def alloc_xblock(b):
                xts[b] = xpool.tile([F_IN, tblk, BL], FP16,
                                    name=f"xb{b}", tag="xblk")
                nc.sync.dma_start(out=xts[b], in_=xd[b])

            alloc_xblock(0)
            alloc_xblock(1)

            def mm_half(z, chain, w, rhs, is_first, is_last):
                """Emit the 4 gate-slot matmuls of one z contributor.
                Within each 2KB PSUM bank the first executed matmul must
                carry start=True and the last stop=True; `chain` pins the
                execution order inside each bank with same-engine deps."""
                for bank0 in range(0, 4, slots_per_bank):
                    bslots = list(range(bank0, min(bank0 + slots_per_bank, 4)))
                    bk = bank0 // slots_per_bank
                    for i, s in enumerate(bslots):
                        mm = nc.tensor.matmul(
                            z[:, s * ch:(s + 1) * ch],
                            w[:, s * U:(s + 1) * U],
                            rhs,
                            start=(is_first and i == 0),
                            stop=(is_last and i == len(bslots) - 1),
                        )
                        prev = chain.get(bk)
                        if prev is not None:
                            tile.add_dep_helper(
                                mm.ins, prev.ins, sync=False,
                                reason="psum bank group order")
                        chain[bk] = mm

            # Per-chunk in-flight z tiles / chains.
            z0t = [None] * nch    # z0 tile consumed by sig0 at step t
            z1t = [None] * nch
            z0n = [None] * nch    # z0 tile being assembled for step t+1
            z0n_chain = [None] * nch

            def start_z0(c, t):
                """Open z0 for step t: x(t) + shadow[acc(t-2)] + rk0(h0(t-1)).
                These matmuls are (nearly) dependency-free, so they double as
                PE keep-warm filler between the chain-critical groups.
                Wh(h1(t-1)) closes the group, adding the missing res(t-1)
                contribution: acc(t-1) = acc(t-2) + res(t-1)."""
                zt = zpool.tile([U, 4 * ch], FP32, name="z0", tag="z")
                chain = {}
                b, j = t // tblk, t % tblk
                lo, hi = c * ch, (c + 1) * ch
                mm_half(zt, chain, kxX, xts[b][:, j, lo:hi], True, False)
                mm_half(zt, chain, kxA, shads[c][(t - 2) % 2], False, False)
                mm_half(zt, chain, rk0, h0s[c], False, t == 0)
                z0n[c] = zt
                z0n_chain[c] = chain

            def finish_z0(c):
                """Close z0(t+1) with the Wh(h1(t)) res-part."""
                mm_half(z0n[c], z0n_chain[c], wh, h1s[c], False, True)
                z0t[c] = z0n[c]
                z0n[c] = None

            # --- per-(chunk, cell) elementwise phases ---
            st = [{}, {}]

            def p_sig(c, cell):
                z = z0t[c] if cell == 0 else z1t[c]
                sio = gates.tile([U, 4 * ch], FP16, name=f"sio{cell}",
                                 tag="sio")
                nc.scalar.activation(sio, z, AF.Sigmoid)
                st[c][f"sio{cell}"] = sio

            def p_dve(c, cell):
                sio = st[c][f"sio{cell}"]
                tgc = (tgc0s if cell == 0 else tgc1s)[c]
                uv = gates.tile([U, 2 * ch], FP16, name="uv", tag="uv")
                # tg = 2*sigmoid(2g) - 1 = tanh(g)
                nc.vector.tensor_scalar(
                    tgc[:, 0:ch], sio[:, 3 * ch:4 * ch],
                    2.0, 1.0, ALU.mult, ALU.subtract)
                # [u|v] = [s_i|s_f] * [tg|c]
                nc.vector.tensor_tensor(
                    out=uv, in0=sio[:, 0:2 * ch], in1=tgc, op=ALU.mult)
                # c_new = u + v  (written into the c half of tgc)
                nc.vector.tensor_tensor(
                    out=tgc[:, ch:2 * ch], in0=uv[:, 0:ch],
                    in1=uv[:, ch:2 * ch], op=ALU.add)

            def p_tanh(c, cell):
                tgc = (tgc0s if cell == 0 else tgc1s)[c]
                tc_t = gates.tile([U, ch], FP16, name="tc", tag="tc")
                nc.scalar.activation(tc_t, tgc[:, ch:2 * ch], AF.Tanh)
                st[c][f"tc{cell}"] = tc_t

            def p_h(c, cell):
                sio = st[c][f"sio{cell}"]
                hstate = (h0s if cell == 0 else h1s)[c]
                nc.vector.tensor_tensor(
                    out=hstate, in0=sio[:, 2 * ch:3 * ch],
                    in1=st[c][f"tc{cell}"], op=ALU.mult)

            def p_z1(c, t):
                """After h0(t): z1(t) = rk1(h1(t-1)) + k1(h0(t)).
                Chain-critical: feeds sig1 of this step."""
                zt = zpool.tile([U, 4 * ch], FP32, name="z1", tag="z")
                chain = {}
                mm_half(zt, chain, rk1, h1s[c], True, False)
                mm_half(zt, chain, k1, h0s[c], False, True)
                z1t[c] = zt

            def p_wh(c, t):
                """After h1(t): close z0(t+1) with Wh (chain-critical) and
                fold res into acc via wout (off-chain)."""
                if t + 1 < T:
                    finish_z0(c)
                nc.tensor.matmul(accps[c], wout, h1s[c],
                                 start=(t == 0), stop=True,
                                 skip_group_check=(t > 0))

            def p_tail(c, t):
                """acc snapshot -> fp32 history; acc -> fp16 shadow for the
                step-(t+2) kxA matmuls (cast reads the SBUF history copy,
                which is cheaper than a second PSUM read)."""
                b, j = t // tblk, t % tblk
                lo, hi = c * ch, (c + 1) * ch
                nc.vector.tensor_copy(out=ots[b][:, j, lo:hi], in_=accps[c])
                if t + 2 < T:
                    nc.vector.tensor_copy(
                        out=shads[c][t % 2], in_=ots[b][:, j, lo:hi])

            # --- prologue: z0(0) for both chunks ---
            for c in range(nch):
                start_z0(c, 0)
                z0t[c] = z0n[c]
                z0n[c] = None

            A, B = 0, 1
            for t in range(T):
                b, j = t // tblk, t % tblk
                if j == 0:
                    if b + 2 < nblk and xts[b + 2] is None:
                        alloc_xblock(b + 2)
                    ots[b] = opool.tile([N_OUT, tblk, BL], FP32,
                                        name=f"ob{b}", tag="oblk")
                # slot 1-4: A cell0(t) with B cell1(t-1) trailing
                p_sig(A, 0)
                if t > 0:
                    p_sig(B, 1)
                p_dve(A, 0)
                if t > 0:
                    p_dve(B, 1)
                p_tanh(A, 0)
                if t > 0:
                    p_tanh(B, 1)
                p_h(A, 0)
                if t > 0:
                    p_h(B, 1)
                # slot 5: chain-critical matmul groups first, prefill last
                p_z1(A, t)
                if t > 0:
                    p_wh(B, t - 1)
                    p_tail(B, t - 1)
                    if j == 0:
                        # B's tail for the last row of block b-1 just ran.
                        nc.sync.dma_start(out=accd[b - 1], in_=ots[b - 1])
                if t + 1 < T:
                    start_z0(A, t + 1)
                # slot 6-9: A cell1(t) with B cell0(t)
                p_sig(A, 1)
                p_sig(B, 0)
                p_dve(A, 1)
                p_dve(B, 0)
                p_tanh(A, 1)
                p_tanh(B, 0)
                p_h(A, 1)
                p_h(B, 0)
                # slot 10
                p_wh(A, t)
                p_z1(B, t)
                p_tail(A, t)
                if t + 1 < T:
                    start_z0(B, t + 1)

            # epilogue: B's cell1 of the last step + final DMA
            t = T - 1
            p_sig(B, 1)
            p_dve(B, 1)
            p_tanh(B, 1)
            p_h(B, 1)
            p_wh(B, t)
            p_tail(B, t)
            nc.sync.dma_start(out=accd[nblk - 1], in_=ots[nblk - 1])

    nc.compile()
    meta = dict(T=T, BL=BL, tblk=tblk, nblk=nblk, nch=nch, ch=ch)
    return nc, meta


# Column reorder: reference gate order in z is [i, f, g, o]; PSUM slot
# order is [i, f, o, g] so sigmoid covers slots 0..2 contiguously and g
# sits in the last slot for the tg fix-up.
def _reorder_cols(w):
    u = w.shape[1] // 4
    return np.concatenate(
        [w[:, 0:u], w[:, u:2 * u], w[:, 3 * u:4 * u], w[:, 2 * u:3 * u]], axis=1)


def _prep_z_weight(w):
    """Reorder to [i,f,o,g] and scale the g columns by 2 (sigmoid trick)."""
    w = _reorder_cols(w)
    u = w.shape[1] // 4
    w = w.copy()
    w[:, 3 * u:] *= 2.0
    return w.astype(np.float16)


def prep_weights(W_in, b_in, k0, rk0, bb0, k1, rk1, bb1, W_out, b_out):
    assert np.allclose(b_in, 0) and np.allclose(bb0, 0) and np.allclose(bb1, 0), \
        "nonzero ann/lstm biases not supported by this kernel build"
    k0f = np.asarray(k0, dtype=np.float64)
    Wf = np.asarray(W_in, dtype=np.float64)
    kxX = (Wf[:F_IN] @ k0f).astype(np.float32)          # [64, 512]
    kxA = (Wf[F_IN:] @ k0f).astype(np.float32)          # [32, 512]
    Wh = (np.asarray(W_out, np.float64) @ (Wf[F_IN:] @ k0f)).astype(np.float32)
    return {
        "kxX": _prep_z_weight(kxX),
        "kxA": _prep_z_weight(kxA),
        "wh": _prep_z_weight(Wh),
        "rk0": _prep_z_weight(np.asarray(rk0, np.float32)),
        "k1": _prep_z_weight(np.asarray(k1, np.float32)),
        "rk1": _prep_z_weight(np.asarray(rk1, np.float32)),
        "wout": np.asarray(W_out).astype(np.float16),
    }


def prep_x_core(x_core, tblk):
    """[BL, T, F] fp32 -> [nblk, F, tblk, BL] fp16."""
    BL, T, F = x_core.shape
    nblk = T // tblk
    xt = np.ascontiguousarray(x_core.transpose(1, 2, 0))       # [T, F, BL]
    xt = xt.reshape(nblk, tblk, F, BL).transpose(0, 2, 1, 3)   # [nblk,F,tblk,BL]
    return np.ascontiguousarray(xt).astype(np.float16)


def post_acc_core(acc_hist, b_out):
    """[nblk, 32, tblk, BL] fp32 acc history -> [BL, T, 32] res."""
    nblk, n_out, tblk, BL = acc_hist.shape
    acc = acc_hist.transpose(0, 2, 3, 1).reshape(nblk * tblk, BL, n_out)
    res = np.empty_like(acc)
    res[0] = acc[0]
    np.subtract(acc[1:], acc[:-1], out=res[1:])
    out = res.transpose(1, 0, 2) + b_out.astype(np.float32)
    return np.ascontiguousarray(out.astype(np.float32))


def kernel(inputs, W_in, b_in, k0, rk0, bb0, k1, rk1, bb1, W_out, b_out):
    inputs = np.asarray(inputs, dtype=np.float32)
    W_in, b_in, k0, rk0, bb0, k1, rk1, bb1, W_out, b_out = (
        np.asarray(a, dtype=np.float32)
        for a in (W_in, b_in, k0, rk0, bb0, k1, rk1, bb1, W_out, b_out))
    weights = prep_weights(W_in, b_in, k0, rk0, bb0, k1, rk1, bb1, W_out, b_out)

    nc, meta = build_program()
    in_maps = []
    for r in range(NCORES):
        x_core = inputs[r * BL_FULL:(r + 1) * BL_FULL]
        m = dict(weights)
        m["x"] = prep_x_core(x_core, meta["tblk"])
        in_maps.append(m)

    ret = run_bass_kernel_spmd(nc, in_maps, core_ids=list(range(NCORES)),
                               trace=TRACE)
    if TRACE:
        print("exec_time_ns:", ret.exec_time_ns,
              "mean:", ret.mean_exec_time_ns)
        if ret.instructions_and_trace is not None:
            print("trace:", ret.instructions_and_trace[1])
        kernel.last_results = ret

    out = np.empty((B_FULL, T_FULL, N_OUT), dtype=np.float32)
    for r in range(NCORES):
        out[r * BL_FULL:(r + 1) * BL_FULL] = post_acc_core(
            ret.results[r]["acc_hist"], np.asarray(b_out))
    return out


# revision 21
# speedup vs baseline: 1.0204x; 1.0056x over previous
"""Trainium2 Bass kernel for nn_AccLSTMCell (v2).

Model (per time step t, per batch row):
    a   = concat(x_t, acc) @ W_in + b_in            (linear)
    h0,c0 = LSTMCell(a,  h0, c0; k0, rk0, bb0)
    h1,c1 = LSTMCell(h0, h1, c1; k1, rk1, bb1)
    res = h1 @ W_out + b_out
    acc = acc + res ;  output[t] = res

Key structure (v2):
  * Data parallel: batch 4096 -> 512 rows per NeuronCore (8 cores),
    each core splits its 512 rows into 2 chunks of 256 for pipelining.
  * Layout: features/units on SBUF partitions, batch on free dim.
  * W_in folded into k0 host-side and split into three z0 contributors:
      x-part   Wx  = W_in[:64]  @ k0   (K=64,  dep-free)
      acc-part Wa  = W_in[64:96]@ k0   (K=32,  reads fp16 acc shadow)
      res-part Wh  = W_out @ Wa        (K=128, reads h1 directly)
    z0(t+1) = x(t+1)@Wx + shadow(t-1)@Wa + h0(t)@rk0 + h1(t)@Wh.
    The Wh path keeps the per-step wout->cast sequence OFF the critical
    recurrence chain (the acc shadow cast has a full step of slack).
  * All four gates go through ONE sigmoid per cell: the g columns of
    every z-contributing weight are pre-scaled by 2 host-side, so
    sigmoid(2g) = (tanh(g)+1)/2 and the DVE fixes tg = 2*s_g - 1.
  * fp16 matmuls (fp32 PSUM), fp16 gates/h/c, fp32 acc in PSUM
    (accumulated for free by the wout matmul, start=False).
  * Kernel emits acc history; host recovers res[t] = acc[t]-acc[t-1].
  * Emission interleaves the two chunks at CELL granularity (chunk B
    lags chunk A by one cell) so each engine always has the other
    chunk's ready work behind the current op (hides cross-engine
    semaphore latency; keeps the PE dense so HAM stays at 2.4 GHz).
"""

import os

import numpy as np

import concourse.bass as bass
import concourse.bacc as bacc
import concourse.tile as tile
from concourse import mybir
from concourse.bass_utils import run_bass_kernel_spmd

# Problem constants (hardcoded; harness contract).
B_FULL, T_FULL, F_IN = 4096, 256, 64
N_OUT = 32
ANN = 128          # ann_in dense width
U = 128            # units of both LSTM cells
NCORES = 8
BL_FULL = B_FULL // NCORES   # 512 batch rows per core

FP16 = mybir.dt.float16
FP32 = mybir.dt.float32
AF = mybir.ActivationFunctionType
ALU = mybir.AluOpType

TBLK = int(os.environ.get("LSTM_TBLK", "16"))   # time steps per DMA block
NCH = 2                                          # batch chunks per core
ZBUFS = int(os.environ.get("LSTM_ZBUFS", "3"))
GBUFS = int(os.environ.get("LSTM_GBUFS", "3"))
TRACE = os.environ.get("LSTM_TRACE", "0") == "1"

PSUM_BANK_BYTES = 2048


def build_program(T=T_FULL, BL=BL_FULL, tblk=TBLK, nch=NCH):
    """Build the per-core Bass program. Returns (nc, meta)."""
    assert T % tblk == 0 and BL % nch == 0
    nblk = T // tblk
    ch = BL // nch
    assert nch == 2, "emission interleave assumes two chunks"

    nc = bacc.Bacc("TRN2", name="acclstm2")

    xd = nc.dram_tensor("x", [nblk, F_IN, tblk, BL], FP16, kind="ExternalInput")
    accd = nc.dram_tensor("acc_hist", [nblk, N_OUT, tblk, BL], FP32,
                          kind="ExternalOutput")
    # Weights: column order is PSUM-slot order [i, f, o, g] (host reorders;
    # g columns pre-scaled by 2 for the sigmoid-only gate trick).
    kxXd = nc.dram_tensor("kxX", [F_IN, 4 * U], FP16, kind="ExternalInput")
    kxAd = nc.dram_tensor("kxA", [N_OUT, 4 * U], FP16, kind="ExternalInput")
    whd = nc.dram_tensor("wh", [U, 4 * U], FP16, kind="ExternalInput")
    rk0d = nc.dram_tensor("rk0", [U, 4 * U], FP16, kind="ExternalInput")
    k1d = nc.dram_tensor("k1", [U, 4 * U], FP16, kind="ExternalInput")
    rk1d = nc.dram_tensor("rk1", [U, 4 * U], FP16, kind="ExternalInput")
    woutd = nc.dram_tensor("wout", [U, N_OUT], FP16, kind="ExternalInput")

    slots_per_bank = max(1, PSUM_BANK_BYTES // (ch * 4))

    with tile.TileContext(nc) as tc:
        with (
            tc.tile_pool(name="wpool", bufs=1) as wpool,
            tc.tile_pool(name="state", bufs=1) as state,
            tc.tile_pool(name="xpool", bufs=2) as xpool,
            tc.tile_pool(name="opool", bufs=2) as opool,
            tc.tile_pool(name="gates", bufs=GBUFS) as gates,
            tc.tile_pool(name="zpool", bufs=ZBUFS, space="PSUM") as zpool,
            tc.tile_pool(name="apool", bufs=1, space="PSUM") as apool,
        ):
            # --- weights -> SBUF ---
            def wload(nm, dram, shape):
                t_ = wpool.tile(shape, FP16, name=nm, tag=nm)
                nc.sync.dma_start(out=t_, in_=dram[:, :])
                return t_
            kxX = wload("kxX", kxXd, [F_IN, 4 * U])
            kxA = wload("kxA", kxAd, [N_OUT, 4 * U])
            wh = wload("wh", whd, [U, 4 * U])
            rk0 = wload("rk0", rk0d, [U, 4 * U])
            k1 = wload("k1", k1d, [U, 4 * U])
            rk1 = wload("rk1", rk1d, [U, 4 * U])
            wout = wload("wout", woutd, [U, N_OUT])

            # --- persistent per-chunk state ---
            h0s, h1s, tgc0s, tgc1s, shads = [], [], [], [], []
            for c in range(nch):
                h0 = state.tile([U, ch], FP16, name=f"h0_{c}", tag=f"h0_{c}")
                h1 = state.tile([U, ch], FP16, name=f"h1_{c}", tag=f"h1_{c}")
                # [tg | c] combined tile per cell: tg scratch, c persistent.
                tgc0 = state.tile([U, 2 * ch], FP16, name=f"tgc0_{c}",
                                  tag=f"tgc0_{c}")
                tgc1 = state.tile([U, 2 * ch], FP16, name=f"tgc1_{c}",
                                  tag=f"tgc1_{c}")
                sh = [state.tile([N_OUT, ch], FP16, name=f"sh{k}_{c}",
                                 tag=f"sh{k}_{c}") for k in range(2)]
                for tl in (h0, h1, tgc0, tgc1, sh[0], sh[1]):
                    nc.vector.memset(tl, 0.0)
                h0s.append(h0); h1s.append(h1)
                tgc0s.append(tgc0); tgc1s.append(tgc1); shads.append(sh)

            # fp32 acc accumulators live in PSUM, one bank per chunk,
            # updated by the wout matmul itself (start=False accumulate).
            accps = [apool.tile([N_OUT, ch], FP32, name=f"accps{c}",
                                tag=f"accps{c}") for c in range(nch)]

            # --- x input blocks / output history blocks ---
            xts = [None] * nblk
            ots = [None] * nblk

            def alloc_xblock(b):
                xts[b] = xpool.tile([F_IN, tblk, BL], FP16,
                                    name=f"xb{b}", tag="xblk")
                nc.sync.dma_start(out=xts[b], in_=xd[b])

            alloc_xblock(0)
            alloc_xblock(1)

            def mm_half(z, chain, w, rhs, is_first, is_last):
                """Emit the 4 gate-slot matmuls of one z contributor.
                Within each 2KB PSUM bank the first executed matmul must
                carry start=True and the last stop=True; `chain` pins the
                execution order inside each bank with same-engine deps."""
                for bank0 in range(0, 4, slots_per_bank):
                    bslots = list(range(bank0, min(bank0 + slots_per_bank, 4)))
                    bk = bank0 // slots_per_bank
                    for i, s in enumerate(bslots):
                        mm = nc.tensor.matmul(
                            z[:, s * ch:(s + 1) * ch],
                            w[:, s * U:(s + 1) * U],
                            rhs,
                            start=(is_first and i == 0),
                            stop=(is_last and i == len(bslots) - 1),
                        )
                        prev = chain.get(bk)
                        if prev is not None:
                            tile.add_dep_helper(
                                mm.ins, prev.ins, sync=False,
                                reason="psum bank group order")
                        chain[bk] = mm

            # Per-chunk in-flight z tiles / chains.
            z0t = [None] * nch    # z0 tile consumed by sig0 at step t
            z1t = [None] * nch
            z0n = [None] * nch    # z0 tile being assembled for step t+1
            z0n_chain = [None] * nch

            def start_z0(c, t):
                """Open z0 for step t: x(t) + shadow[acc(t-2)] + rk0(h0(t-1)).
                These matmuls are (nearly) dependency-free, so they double as
                PE keep-warm filler between the chain-critical groups.
                Wh(h1(t-1)) closes the group, adding the missing res(t-1)
                contribution: acc(t-1) = acc(t-2) + res(t-1)."""
                zt = zpool.tile([U, 4 * ch], FP32, name="z0", tag="z")
                chain = {}
                b, j = t // tblk, t % tblk
                lo, hi = c * ch, (c + 1) * ch
                mm_half(zt, chain, kxX, xts[b][:, j, lo:hi], True, False)
                mm_half(zt, chain, kxA, shads[c][(t - 2) % 2], False, False)
                mm_half(zt, chain, rk0, h0s[c], False, t == 0)
                z0n[c] = zt
                z0n_chain[c] = chain

            def finish_z0(c):
                """Close z0(t+1) with the Wh(h1(t)) res-part."""
                mm_half(z0n[c], z0n_chain[c], wh, h1s[c], False, True)
                z0t[c] = z0n[c]
                z0n[c] = None

            # --- per-(chunk, cell) elementwise phases ---
            st = [{}, {}]

            def p_sig(c, cell):
                z = z0t[c] if cell == 0 else z1t[c]
                sio = gates.tile([U, 4 * ch], FP16, name=f"sio{cell}",
                                 tag="sio")
                nc.scalar.activation(sio, z, AF.Sigmoid)
                st[c][f"sio{cell}"] = sio

            def p_dve(c, cell):
                sio = st[c][f"sio{cell}"]
                tgc = (tgc0s if cell == 0 else tgc1s)[c]
                uv = gates.tile([U, 2 * ch], FP16, name="uv", tag="uv")
                # tg = 2*sigmoid(2g) - 1 = tanh(g)
                nc.vector.tensor_scalar(
                    tgc[:, 0:ch], sio[:, 3 * ch:4 * ch],
                    2.0, 1.0, ALU.mult, ALU.subtract)
                # [u|v] = [s_i|s_f] * [tg|c]
                nc.vector.tensor_tensor(
                    out=uv, in0=sio[:, 0:2 * ch], in1=tgc, op=ALU.mult)
                # c_new = u + v  (written into the c half of tgc)
                nc.vector.tensor_tensor(
                    out=tgc[:, ch:2 * ch], in0=uv[:, 0:ch],
                    in1=uv[:, ch:2 * ch], op=ALU.add)

            def p_tanh(c, cell):
                tgc = (tgc0s if cell == 0 else tgc1s)[c]
                tc_t = gates.tile([U, ch], FP16, name="tc", tag="tc")
                nc.scalar.activation(tc_t, tgc[:, ch:2 * ch], AF.Tanh)
                st[c][f"tc{cell}"] = tc_t

            def p_h(c, cell):
                sio = st[c][f"sio{cell}"]
                hstate = (h0s if cell == 0 else h1s)[c]
                nc.vector.tensor_tensor(
                    out=hstate, in0=sio[:, 2 * ch:3 * ch],
                    in1=st[c][f"tc{cell}"], op=ALU.mult)

            def p_z1(c, t):
                """After h0(t): z1(t) = rk1(h1(t-1)) + k1(h0(t)).
                Chain-critical: feeds sig1 of this step."""
                zt = zpool.tile([U, 4 * ch], FP32, name="z1", tag="z")
                chain = {}
                mm_half(zt, chain, rk1, h1s[c], True, False)
                mm_half(zt, chain, k1, h0s[c], False, True)
                z1t[c] = zt

            def p_wh(c, t):
                """After h1(t): close z0(t+1) with Wh (chain-critical) and
                fold res into acc via wout (off-chain)."""
                if t + 1 < T:
                    finish_z0(c)
                nc.tensor.matmul(accps[c], wout, h1s[c],
                                 start=(t == 0), stop=True,
                                 skip_group_check=(t > 0))

            def p_tail(c, t):
                """acc snapshot -> fp32 history; acc -> fp16 shadow for the
                step-(t+2) kxA matmuls (cast reads the SBUF history copy,
                which is cheaper than a second PSUM read)."""
                b, j = t // tblk, t % tblk
                lo, hi = c * ch, (c + 1) * ch
                nc.vector.tensor_copy(out=ots[b][:, j, lo:hi], in_=accps[c])
                if t + 2 < T:
                    nc.vector.tensor_copy(
                        out=shads[c][t % 2], in_=ots[b][:, j, lo:hi])

            # --- prologue: z0(0) for both chunks ---
            for c in range(nch):
                start_z0(c, 0)
                z0t[c] = z0n[c]
                z0n[c] = None

            A, B = 0, 1
            for t in range(T):
                b, j = t // tblk, t % tblk
                if j == 0:
                    if b + 2 < nblk and xts[b + 2] is None:
                        alloc_xblock(b + 2)
                    ots[b] = opool.tile([N_OUT, tblk, BL], FP32,
                                        name=f"ob{b}", tag="oblk")
                # slot 1-4: A cell0(t) with B cell1(t-1) trailing
                p_sig(A, 0)
                if t > 0:
                    p_sig(B, 1)
                p_dve(A, 0)
                if t > 0:
                    p_dve(B, 1)
                p_tanh(A, 0)
                if t > 0:
                    p_tanh(B, 1)
                p_h(A, 0)
                if t > 0:
                    p_h(B, 1)
                # slot 5
                p_z1(A, t)
                if t + 1 < T:
                    start_z0(A, t + 1)
                if t > 0:
                    p_wh(B, t - 1)
                    p_tail(B, t - 1)
                    if j == 0:
                        # B's tail for the last row of block b-1 just ran.
                        nc.sync.dma_start(out=accd[b - 1], in_=ots[b - 1])
                # slot 6-9: A cell1(t) with B cell0(t)
                p_sig(A, 1)
                p_sig(B, 0)
                p_dve(A, 1)
                p_dve(B, 0)
                p_tanh(A, 1)
                p_tanh(B, 0)
                p_h(A, 1)
                p_h(B, 0)
                # slot 10
                p_wh(A, t)
                p_tail(A, t)
                p_z1(B, t)
                if t + 1 < T:
                    start_z0(B, t + 1)

            # epilogue: B's cell1 of the last step + final DMA
            t = T - 1
            p_sig(B, 1)
            p_dve(B, 1)
            p_tanh(B, 1)
            p_h(B, 1)
            p_wh(B, t)
            p_tail(B, t)
            nc.sync.dma_start(out=accd[nblk - 1], in_=ots[nblk - 1])

    nc.compile()
    meta = dict(T=T, BL=BL, tblk=tblk, nblk=nblk, nch=nch, ch=ch)
    return nc, meta


# Column reorder: reference gate order in z is [i, f, g, o]; PSUM slot
# order is [i, f, o, g] so sigmoid covers slots 0..2 contiguously and g
# sits in the last slot for the tg fix-up.
def _reorder_cols(w):
    u = w.shape[1] // 4
    return np.concatenate(
        [w[:, 0:u], w[:, u:2 * u], w[:, 3 * u:4 * u], w[:, 2 * u:3 * u]], axis=1)


def _prep_z_weight(w):
    """Reorder to [i,f,o,g] and scale the g columns by 2 (sigmoid trick)."""
    w = _reorder_cols(w)
    u = w.shape[1] // 4
    w = w.copy()
    w[:, 3 * u:] *= 2.0
    return w.astype(np.float16)


def prep_weights(W_in, b_in, k0, rk0, bb0, k1, rk1, bb1, W_out, b_out):
    assert np.allclose(b_in, 0) and np.allclose(bb0, 0) and np.allclose(bb1, 0), \
        "nonzero ann/lstm biases not supported by this kernel build"
    k0f = np.asarray(k0, dtype=np.float64)
    Wf = np.asarray(W_in, dtype=np.float64)
    kxX = (Wf[:F_IN] @ k0f).astype(np.float32)          # [64, 512]
    kxA = (Wf[F_IN:] @ k0f).astype(np.float32)          # [32, 512]
    Wh = (np.asarray(W_out, np.float64) @ (Wf[F_IN:] @ k0f)).astype(np.float32)
    return {
        "kxX": _prep_z_weight(kxX),
        "kxA": _prep_z_weight(kxA),
        "wh": _prep_z_weight(Wh),
        "rk0": _prep_z_weight(np.asarray(rk0, np.float32)),
        "k1": _prep_z_weight(np.asarray(k1, np.float32)),
        "rk1": _prep_z_weight(np.asarray(rk1, np.float32)),
        "wout": np.asarray(W_out).astype(np.float16),
    }


def prep_x_core(x_core, tblk):
    """[BL, T, F] fp32 -> [nblk, F, tblk, BL] fp16."""
    BL, T, F = x_core.shape
    nblk = T // tblk
    xt = np.ascontiguousarray(x_core.transpose(1, 2, 0))       # [T, F, BL]
    xt = xt.reshape(nblk, tblk, F, BL).transpose(0, 2, 1, 3)   # [nblk,F,tblk,BL]
    return np.ascontiguousarray(xt).astype(np.float16)


def post_acc_core(acc_hist, b_out):
    """[nblk, 32, tblk, BL] fp32 acc history -> [BL, T, 32] res."""
    nblk, n_out, tblk, BL = acc_hist.shape
    acc = acc_hist.transpose(0, 2, 3, 1).reshape(nblk * tblk, BL, n_out)
    res = np.empty_like(acc)
    res[0] = acc[0]
    np.subtract(acc[1:], acc[:-1], out=res[1:])
    out = res.transpose(1, 0, 2) + b_out.astype(np.float32)
    return np.ascontiguousarray(out.astype(np.float32))


def kernel(inputs, W_in, b_in, k0, rk0, bb0, k1, rk1, bb1, W_out, b_out):
    inputs = np.asarray(inputs, dtype=np.float32)
    W_in, b_in, k0, rk0, bb0, k1, rk1, bb1, W_out, b_out = (
        np.asarray(a, dtype=np.float32)
        for a in (W_in, b_in, k0, rk0, bb0, k1, rk1, bb1, W_out, b_out))
    weights = prep_weights(W_in, b_in, k0, rk0, bb0, k1, rk1, bb1, W_out, b_out)

    nc, meta = build_program()
    in_maps = []
    for r in range(NCORES):
        x_core = inputs[r * BL_FULL:(r + 1) * BL_FULL]
        m = dict(weights)
        m["x"] = prep_x_core(x_core, meta["tblk"])
        in_maps.append(m)

    ret = run_bass_kernel_spmd(nc, in_maps, core_ids=list(range(NCORES)),
                               trace=TRACE)
    if TRACE:
        print("exec_time_ns:", ret.exec_time_ns,
              "mean:", ret.mean_exec_time_ns)
        if ret.instructions_and_trace is not None:
            print("trace:", ret.instructions_and_trace[1])
        kernel.last_results = ret

    out = np.empty((B_FULL, T_FULL, N_OUT), dtype=np.float32)
    for r in range(NCORES):
        out[r * BL_FULL:(r + 1) * BL_FULL] = post_acc_core(
            ret.results[r]["acc_hist"], np.asarray(b_out))
    return out


# revision 23
# speedup vs baseline: 1.0988x; 1.0768x over previous
"""Trainium2 Bass kernel for nn_AccLSTMCell (v2).

Model (per time step t, per batch row):
    a   = concat(x_t, acc) @ W_in + b_in            (linear)
    h0,c0 = LSTMCell(a,  h0, c0; k0, rk0, bb0)
    h1,c1 = LSTMCell(h0, h1, c1; k1, rk1, bb1)
    res = h1 @ W_out + b_out
    acc = acc + res ;  output[t] = res

Key structure (v2):
  * Data parallel: batch 4096 -> 512 rows per NeuronCore (8 cores),
    each core splits its 512 rows into 2 chunks of 256 for pipelining.
  * Layout: features/units on SBUF partitions, batch on free dim.
  * W_in folded into k0 host-side and split into three z0 contributors:
      x-part   Wx  = W_in[:64]  @ k0   (K=64,  dep-free)
      acc-part Wa  = W_in[64:96]@ k0   (K=32,  reads fp16 acc shadow)
      res-part Wh  = W_out @ Wa        (K=128, reads h1 directly)
    z0(t+1) = x(t+1)@Wx + shadow(t-1)@Wa + h0(t)@rk0 + h1(t)@Wh.
    The Wh path keeps the per-step wout->cast sequence OFF the critical
    recurrence chain (the acc shadow cast has a full step of slack).
  * All four gates go through ONE sigmoid per cell: the g columns of
    every z-contributing weight are pre-scaled by 2 host-side, so
    sigmoid(2g) = (tanh(g)+1)/2 and the DVE fixes tg = 2*s_g - 1.
  * fp16 matmuls (fp32 PSUM), fp16 gates/h/c, fp32 acc in PSUM
    (accumulated for free by the wout matmul, start=False).
  * Kernel emits acc history; host recovers res[t] = acc[t]-acc[t-1].
  * Emission interleaves the two chunks at CELL granularity (chunk B
    lags chunk A by one cell) so each engine always has the other
    chunk's ready work behind the current op (hides cross-engine
    semaphore latency; keeps the PE dense so HAM stays at 2.4 GHz).
"""

import os

import numpy as np

import concourse.bass as bass
import concourse.bacc as bacc
import concourse.tile as tile
from concourse import mybir
from concourse.bass_utils import run_bass_kernel_spmd

# Problem constants (hardcoded; harness contract).
B_FULL, T_FULL, F_IN = 4096, 256, 64
N_OUT = 32
ANN = 128          # ann_in dense width
U = 128            # units of both LSTM cells
NCORES = 8
BL_FULL = B_FULL // NCORES   # 512 batch rows per core

FP16 = mybir.dt.float16
FP32 = mybir.dt.float32
AF = mybir.ActivationFunctionType
ALU = mybir.AluOpType

TBLK = int(os.environ.get("LSTM_TBLK", "16"))   # time steps per DMA block
NCH = 2                                          # batch chunks per core
ZBUFS = int(os.environ.get("LSTM_ZBUFS", "3"))
GBUFS = int(os.environ.get("LSTM_GBUFS", "3"))
TRACE = os.environ.get("LSTM_TRACE", "0") == "1"

PSUM_BANK_BYTES = 2048


def build_program(T=T_FULL, BL=BL_FULL, tblk=TBLK, nch=NCH):
    """Build the per-core Bass program. Returns (nc, meta)."""
    assert T % tblk == 0 and BL % nch == 0
    nblk = T // tblk
    ch = BL // nch
    assert nch == 2, "emission interleave assumes two chunks"

    nc = bacc.Bacc("TRN2", name="acclstm2")

    xd = nc.dram_tensor("x", [nblk, F_IN, tblk, BL], FP16, kind="ExternalInput")
    accd = nc.dram_tensor("acc_hist", [nblk, N_OUT, tblk, BL], FP32,
                          kind="ExternalOutput")
    # Weights: column order is PSUM-slot order [i, f, o, g] (host reorders;
    # g columns pre-scaled by 2 for the sigmoid-only gate trick).
    kxXd = nc.dram_tensor("kxX", [F_IN, 4 * U], FP16, kind="ExternalInput")
    kxAd = nc.dram_tensor("kxA", [N_OUT, 4 * U], FP16, kind="ExternalInput")
    whd = nc.dram_tensor("wh", [U, 4 * U], FP16, kind="ExternalInput")
    rk0d = nc.dram_tensor("rk0", [U, 4 * U], FP16, kind="ExternalInput")
    k1d = nc.dram_tensor("k1", [U, 4 * U], FP16, kind="ExternalInput")
    rk1d = nc.dram_tensor("rk1", [U, 4 * U], FP16, kind="ExternalInput")
    woutd = nc.dram_tensor("wout", [U, N_OUT], FP16, kind="ExternalInput")

    slots_per_bank = max(1, PSUM_BANK_BYTES // (ch * 4))

    with tile.TileContext(nc) as tc:
        with (
            tc.tile_pool(name="wpool", bufs=1) as wpool,
            tc.tile_pool(name="state", bufs=1) as state,
            tc.tile_pool(name="xpool", bufs=2) as xpool,
            tc.tile_pool(name="opool", bufs=2) as opool,
            tc.tile_pool(name="gates", bufs=GBUFS) as gates,
            tc.tile_pool(name="zpool", bufs=ZBUFS, space="PSUM") as zpool,
            tc.tile_pool(name="apool", bufs=1, space="PSUM") as apool,
        ):
            # --- weights -> SBUF ---
            def wload(nm, dram, shape):
                t_ = wpool.tile(shape, FP16, name=nm, tag=nm)
                nc.sync.dma_start(out=t_, in_=dram[:, :])
                return t_
            kxX = wload("kxX", kxXd, [F_IN, 4 * U])
            kxA = wload("kxA", kxAd, [N_OUT, 4 * U])
            wh = wload("wh", whd, [U, 4 * U])
            rk0 = wload("rk0", rk0d, [U, 4 * U])
            k1 = wload("k1", k1d, [U, 4 * U])
            rk1 = wload("rk1", rk1d, [U, 4 * U])
            wout = wload("wout", woutd, [U, N_OUT])

            # --- persistent per-chunk state ---
            h0s, h1s, tgc0s, tgc1s, shads = [], [], [], [], []
            for c in range(nch):
                h0 = state.tile([U, ch], FP16, name=f"h0_{c}", tag=f"h0_{c}")
                h1 = state.tile([U, ch], FP16, name=f"h1_{c}", tag=f"h1_{c}")
                # [tg | c] combined tile per cell: tg scratch, c persistent.
                tgc0 = state.tile([U, 2 * ch], FP16, name=f"tgc0_{c}",
                                  tag=f"tgc0_{c}")
                tgc1 = state.tile([U, 2 * ch], FP16, name=f"tgc1_{c}",
                                  tag=f"tgc1_{c}")
                sh = [state.tile([N_OUT, ch], FP16, name=f"sh{k}_{c}",
                                 tag=f"sh{k}_{c}") for k in range(2)]
                for tl in (h0, h1, tgc0, tgc1, sh[0], sh[1]):
                    nc.vector.memset(tl, 0.0)
                h0s.append(h0); h1s.append(h1)
                tgc0s.append(tgc0); tgc1s.append(tgc1); shads.append(sh)

            # fp32 acc accumulators live in PSUM, one bank per chunk,
            # updated by the wout matmul itself (start=False accumulate).
            accps = [apool.tile([N_OUT, ch], FP32, name=f"accps{c}",
                                tag=f"accps{c}") for c in range(nch)]

            # --- x input blocks / output history blocks ---
            xts = [None] * nblk
            ots = [None] * nblk

            def alloc_xblock(b):
                xts[b] = xpool.tile([F_IN, tblk, BL], FP16,
                                    name=f"xb{b}", tag="xblk")
                nc.sync.dma_start(out=xts[b], in_=xd[b])

            alloc_xblock(0)
            alloc_xblock(1)

            def mm_half(z, chain, w, rhs, is_first, is_last):
                """Emit the 4 gate-slot matmuls of one z contributor.
                Within each 2KB PSUM bank the first executed matmul must
                carry start=True and the last stop=True; `chain` pins the
                execution order inside each bank with same-engine deps."""
                for bank0 in range(0, 4, slots_per_bank):
                    bslots = list(range(bank0, min(bank0 + slots_per_bank, 4)))
                    bk = bank0 // slots_per_bank
                    for i, s in enumerate(bslots):
                        mm = nc.tensor.matmul(
                            z[:, s * ch:(s + 1) * ch],
                            w[:, s * U:(s + 1) * U],
                            rhs,
                            start=(is_first and i == 0),
                            stop=(is_last and i == len(bslots) - 1),
                        )
                        prev = chain.get(bk)
                        if prev is not None:
                            tile.add_dep_helper(
                                mm.ins, prev.ins, sync=False,
                                reason="psum bank group order")
                        chain[bk] = mm

            # Per-chunk in-flight z tiles / chains.
            z0t = [None] * nch    # z0 tile consumed by sig0 at step t
            z1t = [None] * nch
            z0n = [None] * nch    # z0 tile being assembled for step t+1
            z0n_chain = [None] * nch

            def start_z0(c, t):
                """Open z0 for step t: x(t) + shadow[acc(t-2)] + rk0(h0(t-1)).
                These matmuls are (nearly) dependency-free, so they double as
                PE keep-warm filler between the chain-critical groups.
                Wh(h1(t-1)) closes the group, adding the missing res(t-1)
                contribution: acc(t-1) = acc(t-2) + res(t-1)."""
                zt = zpool.tile([U, 4 * ch], FP32, name="z0", tag="z")
                chain = {}
                b, j = t // tblk, t % tblk
                lo, hi = c * ch, (c + 1) * ch
                mm_half(zt, chain, kxX, xts[b][:, j, lo:hi], True, False)
                mm_half(zt, chain, kxA, shads[c][(t - 2) % 2], False, False)
                mm_half(zt, chain, rk0, h0s[c], False, t == 0)
                z0n[c] = zt
                z0n_chain[c] = chain

            def finish_z0(c):
                """Close z0(t+1) with the Wh(h1(t)) res-part."""
                mm_half(z0n[c], z0n_chain[c], wh, h1s[c], False, True)
                z0t[c] = z0n[c]
                z0n[c] = None

            # --- per-(chunk, cell) elementwise phases ---
            st = [{}, {}]

            def p_sig(c, cell):
                z = z0t[c] if cell == 0 else z1t[c]
                sio = gates.tile([U, 4 * ch], FP16, name=f"sio{cell}",
                                 tag="sio")
                nc.scalar.activation(sio, z, AF.Sigmoid)
                st[c][f"sio{cell}"] = sio

            def p_dve(c, cell):
                sio = st[c][f"sio{cell}"]
                tgc = (tgc0s if cell == 0 else tgc1s)[c]
                uv = gates.tile([U, 2 * ch], FP16, name="uv", tag="uv")
                # tg = 2*sigmoid(2g) - 1 = tanh(g)
                nc.vector.tensor_scalar(
                    tgc[:, 0:ch], sio[:, 3 * ch:4 * ch],
                    2.0, 1.0, ALU.mult, ALU.subtract)
                # [u|v] = [s_i|s_f] * [tg|c]
                nc.vector.tensor_tensor(
                    out=uv, in0=sio[:, 0:2 * ch], in1=tgc, op=ALU.mult)
                # c_new = u + v  (written into the c half of tgc)
                nc.vector.tensor_tensor(
                    out=tgc[:, ch:2 * ch], in0=uv[:, 0:ch],
                    in1=uv[:, ch:2 * ch], op=ALU.add)

            def p_tanh(c, cell):
                tgc = (tgc0s if cell == 0 else tgc1s)[c]
                tc_t = gates.tile([U, ch], FP16, name="tc", tag="tc")
                nc.scalar.activation(tc_t, tgc[:, ch:2 * ch], AF.Tanh)
                st[c][f"tc{cell}"] = tc_t

            def p_h(c, cell):
                sio = st[c][f"sio{cell}"]
                hstate = (h0s if cell == 0 else h1s)[c]
                nc.vector.tensor_tensor(
                    out=hstate, in0=sio[:, 2 * ch:3 * ch],
                    in1=st[c][f"tc{cell}"], op=ALU.mult)

            def p_z1_rk(c):
                """rk1(h1(t-1)) half of z1(t) - ready early."""
                zt = zpool.tile([U, 4 * ch], FP32, name="z1", tag="z")
                chain = {}
                mm_half(zt, chain, rk1, h1s[c], True, False)
                z1t[c] = zt
                return chain

            def p_z1_k1(c, chain):
                """k1(h0(t)) half of z1(t) - gated on h0. Emitted AFTER all
                dependency-free matmuls of this slot: the PE executes
                matmuls strictly in order, so anything queued behind this
                would otherwise sit idle while h0 is computed."""
                mm_half(z1t[c], chain, k1, h0s[c], False, True)

            def p_wh(c, t):
                """After h1(t): close z0(t+1) with Wh (chain-critical) and
                fold res into acc via wout (off-chain)."""
                if t + 1 < T:
                    finish_z0(c)
                nc.tensor.matmul(accps[c], wout, h1s[c],
                                 start=(t == 0), stop=True,
                                 skip_group_check=(t > 0))

            def p_tail(c, t):
                """acc snapshot -> fp32 history; acc -> fp16 shadow for the
                step-(t+2) kxA matmuls (cast reads the SBUF history copy,
                which is cheaper than a second PSUM read)."""
                b, j = t // tblk, t % tblk
                lo, hi = c * ch, (c + 1) * ch
                nc.vector.tensor_copy(out=ots[b][:, j, lo:hi], in_=accps[c])
                if t + 2 < T:
                    nc.vector.tensor_copy(out=shads[c][t % 2], in_=accps[c])

            # --- prologue: z0(0) for both chunks ---
            for c in range(nch):
                start_z0(c, 0)
                z0t[c] = z0n[c]
                z0n[c] = None

            A, B = 0, 1
            for t in range(T):
                b, j = t // tblk, t % tblk
                if j == 0:
                    if b + 2 < nblk and xts[b + 2] is None:
                        alloc_xblock(b + 2)
                    ots[b] = opool.tile([N_OUT, tblk, BL], FP32,
                                        name=f"ob{b}", tag="oblk")
                # slot 1-4: A cell0(t) with B cell1(t-1) trailing
                p_sig(A, 0)
                if t > 0:
                    p_sig(B, 1)
                p_dve(A, 0)
                if t > 0:
                    p_dve(B, 1)
                p_tanh(A, 0)
                if t > 0:
                    p_tanh(B, 1)
                p_h(A, 0)
                if t > 0:
                    p_h(B, 1)
                # slot 5
                p_z1(A, t)
                if t + 1 < T:
                    start_z0(A, t + 1)
                if t > 0:
                    p_wh(B, t - 1)
                    p_tail(B, t - 1)
                    if j == 0:
                        # B's tail for the last row of block b-1 just ran.
                        nc.sync.dma_start(out=accd[b - 1], in_=ots[b - 1])
                # slot 6-9: A cell1(t) with B cell0(t)
                p_sig(A, 1)
                p_sig(B, 0)
                p_dve(A, 1)
                p_dve(B, 0)
                p_tanh(A, 1)
                p_tanh(B, 0)
                p_h(A, 1)
                p_h(B, 0)
                # slot 10
                p_wh(A, t)
                p_tail(A, t)
                p_z1(B, t)
                if t + 1 < T:
                    start_z0(B, t + 1)

            # epilogue: B's cell1 of the last step + final DMA
            t = T - 1
            p_sig(B, 1)
            p_dve(B, 1)
            p_tanh(B, 1)
            p_h(B, 1)
            p_wh(B, t)
            p_tail(B, t)
            nc.sync.dma_start(out=accd[nblk - 1], in_=ots[nblk - 1])

    nc.compile()
    meta = dict(T=T, BL=BL, tblk=tblk, nblk=nblk, nch=nch, ch=ch)
    return nc, meta


# Column reorder: reference gate order in z is [i, f, g, o]; PSUM slot
# order is [i, f, o, g] so sigmoid covers slots 0..2 contiguously and g
# sits in the last slot for the tg fix-up.
def _reorder_cols(w):
    u = w.shape[1] // 4
    return np.concatenate(
        [w[:, 0:u], w[:, u:2 * u], w[:, 3 * u:4 * u], w[:, 2 * u:3 * u]], axis=1)


def _prep_z_weight(w):
    """Reorder to [i,f,o,g] and scale the g columns by 2 (sigmoid trick)."""
    w = _reorder_cols(w)
    u = w.shape[1] // 4
    w = w.copy()
    w[:, 3 * u:] *= 2.0
    return w.astype(np.float16)


def prep_weights(W_in, b_in, k0, rk0, bb0, k1, rk1, bb1, W_out, b_out):
    assert np.allclose(b_in, 0) and np.allclose(bb0, 0) and np.allclose(bb1, 0), \
        "nonzero ann/lstm biases not supported by this kernel build"
    k0f = np.asarray(k0, dtype=np.float64)
    Wf = np.asarray(W_in, dtype=np.float64)
    kxX = (Wf[:F_IN] @ k0f).astype(np.float32)          # [64, 512]
    kxA = (Wf[F_IN:] @ k0f).astype(np.float32)          # [32, 512]
    Wh = (np.asarray(W_out, np.float64) @ (Wf[F_IN:] @ k0f)).astype(np.float32)
    return {
        "kxX": _prep_z_weight(kxX),
        "kxA": _prep_z_weight(kxA),
        "wh": _prep_z_weight(Wh),
        "rk0": _prep_z_weight(np.asarray(rk0, np.float32)),
        "k1": _prep_z_weight(np.asarray(k1, np.float32)),
        "rk1": _prep_z_weight(np.asarray(rk1, np.float32)),
        "wout": np.asarray(W_out).astype(np.float16),
    }


def prep_x_core(x_core, tblk):
    """[BL, T, F] fp32 -> [nblk, F, tblk, BL] fp16."""
    BL, T, F = x_core.shape
    nblk = T // tblk
    xt = np.ascontiguousarray(x_core.transpose(1, 2, 0))       # [T, F, BL]
    xt = xt.reshape(nblk, tblk, F, BL).transpose(0, 2, 1, 3)   # [nblk,F,tblk,BL]
    return np.ascontiguousarray(xt).astype(np.float16)


def post_acc_core(acc_hist, b_out):
    """[nblk, 32, tblk, BL] fp32 acc history -> [BL, T, 32] res."""
    nblk, n_out, tblk, BL = acc_hist.shape
    acc = acc_hist.transpose(0, 2, 3, 1).reshape(nblk * tblk, BL, n_out)
    res = np.empty_like(acc)
    res[0] = acc[0]
    np.subtract(acc[1:], acc[:-1], out=res[1:])
    out = res.transpose(1, 0, 2) + b_out.astype(np.float32)
    return np.ascontiguousarray(out.astype(np.float32))


def kernel(inputs, W_in, b_in, k0, rk0, bb0, k1, rk1, bb1, W_out, b_out):
    inputs = np.asarray(inputs, dtype=np.float32)
    W_in, b_in, k0, rk0, bb0, k1, rk1, bb1, W_out, b_out = (
        np.asarray(a, dtype=np.float32)
        for a in (W_in, b_in, k0, rk0, bb0, k1, rk1, bb1, W_out, b_out))
    weights = prep_weights(W_in, b_in, k0, rk0, bb0, k1, rk1, bb1, W_out, b_out)

    nc, meta = build_program()
    in_maps = []
    for r in range(NCORES):
        x_core = inputs[r * BL_FULL:(r + 1) * BL_FULL]
        m = dict(weights)
        m["x"] = prep_x_core(x_core, meta["tblk"])
        in_maps.append(m)

    ret = run_bass_kernel_spmd(nc, in_maps, core_ids=list(range(NCORES)),
                               trace=TRACE)
    if TRACE:
        print("exec_time_ns:", ret.exec_time_ns,
              "mean:", ret.mean_exec_time_ns)
        if ret.instructions_and_trace is not None:
            print("trace:", ret.instructions_and_trace[1])
        kernel.last_results = ret

    out = np.empty((B_FULL, T_FULL, N_OUT), dtype=np.float32)
    for r in range(NCORES):
        out[r * BL_FULL:(r + 1) * BL_FULL] = post_acc_core(
            ret.results[r]["acc_hist"], np.asarray(b_out))
    return out


# revision 27
# speedup vs baseline: 1.0990x; 1.0002x over previous
"""Trainium2 Bass kernel for nn_AccLSTMCell (v2).

Model (per time step t, per batch row):
    a   = concat(x_t, acc) @ W_in + b_in            (linear)
    h0,c0 = LSTMCell(a,  h0, c0; k0, rk0, bb0)
    h1,c1 = LSTMCell(h0, h1, c1; k1, rk1, bb1)
    res = h1 @ W_out + b_out
    acc = acc + res ;  output[t] = res

Key structure (v2):
  * Data parallel: batch 4096 -> 512 rows per NeuronCore (8 cores),
    each core splits its 512 rows into 2 chunks of 256 for pipelining.
  * Layout: features/units on SBUF partitions, batch on free dim.
  * W_in folded into k0 host-side and split into three z0 contributors:
      x-part   Wx  = W_in[:64]  @ k0   (K=64,  dep-free)
      acc-part Wa  = W_in[64:96]@ k0   (K=32,  reads fp16 acc shadow)
      res-part Wh  = W_out @ Wa        (K=128, reads h1 directly)
    z0(t+1) = x(t+1)@Wx + shadow(t-1)@Wa + h0(t)@rk0 + h1(t)@Wh.
    The Wh path keeps the per-step wout->cast sequence OFF the critical
    recurrence chain (the acc shadow cast has a full step of slack).
  * All four gates go through ONE sigmoid per cell: the g columns of
    every z-contributing weight are pre-scaled by 2 host-side, so
    sigmoid(2g) = (tanh(g)+1)/2 and the DVE fixes tg = 2*s_g - 1.
  * fp16 matmuls (fp32 PSUM), fp16 gates/h/c, fp32 acc in PSUM
    (accumulated for free by the wout matmul, start=False).
  * Kernel emits acc history; host recovers res[t] = acc[t]-acc[t-1].
  * Emission interleaves the two chunks at CELL granularity (chunk B
    lags chunk A by one cell) so each engine always has the other
    chunk's ready work behind the current op (hides cross-engine
    semaphore latency; keeps the PE dense so HAM stays at 2.4 GHz).
"""

import os

import numpy as np

import concourse.bass as bass
import concourse.bacc as bacc
import concourse.tile as tile
from concourse import mybir
from concourse.bass_utils import run_bass_kernel_spmd

# Problem constants (hardcoded; harness contract).
B_FULL, T_FULL, F_IN = 4096, 256, 64
N_OUT = 32
ANN = 128          # ann_in dense width
U = 128            # units of both LSTM cells
NCORES = 8
BL_FULL = B_FULL // NCORES   # 512 batch rows per core

FP16 = mybir.dt.float16
FP32 = mybir.dt.float32
AF = mybir.ActivationFunctionType
ALU = mybir.AluOpType

TBLK = int(os.environ.get("LSTM_TBLK", "16"))   # time steps per DMA block
NCH = 2                                          # batch chunks per core
ZBUFS = int(os.environ.get("LSTM_ZBUFS", "3"))
GBUFS = int(os.environ.get("LSTM_GBUFS", "3"))
TRACE = os.environ.get("LSTM_TRACE", "0") == "1"

PSUM_BANK_BYTES = 2048


def build_program(T=T_FULL, BL=BL_FULL, tblk=TBLK, nch=NCH):
    """Build the per-core Bass program. Returns (nc, meta)."""
    assert T % tblk == 0 and BL % nch == 0
    nblk = T // tblk
    ch = BL // nch
    assert nch == 2, "emission interleave assumes two chunks"

    nc = bacc.Bacc("TRN2", name="acclstm2")

    xd = nc.dram_tensor("x", [nblk, F_IN, tblk, BL], FP16, kind="ExternalInput")
    accd = nc.dram_tensor("acc_hist", [nblk, N_OUT, tblk, BL], FP32,
                          kind="ExternalOutput")
    # Weights: column order is PSUM-slot order [i, f, o, g] (host reorders;
    # g columns pre-scaled by 2 for the sigmoid-only gate trick).
    kxXd = nc.dram_tensor("kxX", [F_IN, 4 * U], FP16, kind="ExternalInput")
    kxAd = nc.dram_tensor("kxA", [N_OUT, 4 * U], FP16, kind="ExternalInput")
    whd = nc.dram_tensor("wh", [U, 4 * U], FP16, kind="ExternalInput")
    rk0d = nc.dram_tensor("rk0", [U, 4 * U], FP16, kind="ExternalInput")
    k1d = nc.dram_tensor("k1", [U, 4 * U], FP16, kind="ExternalInput")
    rk1d = nc.dram_tensor("rk1", [U, 4 * U], FP16, kind="ExternalInput")
    woutd = nc.dram_tensor("wout", [U, N_OUT], FP16, kind="ExternalInput")

    slots_per_bank = max(1, PSUM_BANK_BYTES // (ch * 4))

    with tile.TileContext(nc) as tc:
        with (
            tc.tile_pool(name="wpool", bufs=1) as wpool,
            tc.tile_pool(name="state", bufs=1) as state,
            tc.tile_pool(name="xpool", bufs=2) as xpool,
            tc.tile_pool(name="opool", bufs=2) as opool,
            tc.tile_pool(name="gates", bufs=GBUFS) as gates,
            tc.tile_pool(name="zpool", bufs=ZBUFS, space="PSUM") as zpool,
            tc.tile_pool(name="apool", bufs=1, space="PSUM") as apool,
        ):
            # --- weights -> SBUF ---
            def wload(nm, dram, shape):
                t_ = wpool.tile(shape, FP16, name=nm, tag=nm)
                nc.sync.dma_start(out=t_, in_=dram[:, :])
                return t_
            kxX = wload("kxX", kxXd, [F_IN, 4 * U])
            kxA = wload("kxA", kxAd, [N_OUT, 4 * U])
            wh = wload("wh", whd, [U, 4 * U])
            rk0 = wload("rk0", rk0d, [U, 4 * U])
            k1 = wload("k1", k1d, [U, 4 * U])
            rk1 = wload("rk1", rk1d, [U, 4 * U])
            wout = wload("wout", woutd, [U, N_OUT])

            # --- persistent per-chunk state ---
            h0s, h1s, tgc0s, tgc1s, shads = [], [], [], [], []
            for c in range(nch):
                h0 = state.tile([U, ch], FP16, name=f"h0_{c}", tag=f"h0_{c}")
                h1 = state.tile([U, ch], FP16, name=f"h1_{c}", tag=f"h1_{c}")
                # [tg | c] combined tile per cell: tg scratch, c persistent.
                tgc0 = state.tile([U, 2 * ch], FP16, name=f"tgc0_{c}",
                                  tag=f"tgc0_{c}")
                tgc1 = state.tile([U, 2 * ch], FP16, name=f"tgc1_{c}",
                                  tag=f"tgc1_{c}")
                sh = [state.tile([N_OUT, ch], FP16, name=f"sh{k}_{c}",
                                 tag=f"sh{k}_{c}") for k in range(2)]
                for tl in (h0, h1, tgc0, tgc1, sh[0], sh[1]):
                    nc.vector.memset(tl, 0.0)
                h0s.append(h0); h1s.append(h1)
                tgc0s.append(tgc0); tgc1s.append(tgc1); shads.append(sh)

            # fp32 acc accumulators live in PSUM, one bank per chunk,
            # updated by the wout matmul itself (start=False accumulate).
            accps = [apool.tile([N_OUT, ch], FP32, name=f"accps{c}",
                                tag=f"accps{c}") for c in range(nch)]

            # --- x input blocks / output history blocks ---
            xts = [None] * nblk
            ots = [None] * nblk

            def alloc_xblock(b):
                xts[b] = xpool.tile([F_IN, tblk, BL], FP16,
                                    name=f"xb{b}", tag="xblk")
                nc.sync.dma_start(out=xts[b], in_=xd[b])

            alloc_xblock(0)
            alloc_xblock(1)

            def mm_half(z, chain, w, rhs, is_first, is_last):
                """Emit the 4 gate-slot matmuls of one z contributor.
                Within each 2KB PSUM bank the first executed matmul must
                carry start=True and the last stop=True; `chain` pins the
                execution order inside each bank with same-engine deps."""
                for bank0 in range(0, 4, slots_per_bank):
                    bslots = list(range(bank0, min(bank0 + slots_per_bank, 4)))
                    bk = bank0 // slots_per_bank
                    for i, s in enumerate(bslots):
                        mm = nc.tensor.matmul(
                            z[:, s * ch:(s + 1) * ch],
                            w[:, s * U:(s + 1) * U],
                            rhs,
                            start=(is_first and i == 0),
                            stop=(is_last and i == len(bslots) - 1),
                        )
                        prev = chain.get(bk)
                        if prev is not None:
                            tile.add_dep_helper(
                                mm.ins, prev.ins, sync=False,
                                reason="psum bank group order")
                        chain[bk] = mm

            # Per-chunk in-flight z tiles / chains.
            z0t = [None] * nch    # z0 tile consumed by sig0 at step t
            z1t = [None] * nch
            z0n = [None] * nch    # z0 tile being assembled for step t+1
            z0n_chain = [None] * nch

            def start_z0(c, t):
                """Open z0 for step t with the dependency-free parts:
                x(t) + shadow[acc(t-2)]. These execute immediately, keeping
                the PE busy (and HAM warm) while h0/h1 are still being
                computed. rk0(h0(t-1)) and Wh(h1(t-1)) are appended later:
                the PE runs matmuls strictly in order, so gated matmuls
                must be emitted AFTER every ready one."""
                zt = zpool.tile([U, 4 * ch], FP32, name="z0", tag="z")
                chain = {}
                b, j = t // tblk, t % tblk
                lo, hi = c * ch, (c + 1) * ch
                mm_half(zt, chain, kxX, xts[b][:, j, lo:hi], True, False)
                mm_half(zt, chain, kxA, shads[c][(t - 2) % 2], False, False)
                z0n[c] = zt
                z0n_chain[c] = chain

            def z0_rk0(c, t):
                """rk0(h0(t-1)) part of z0(t) - gated on h0."""
                mm_half(z0n[c], z0n_chain[c], rk0, h0s[c], False, t == 0)

            def finish_z0(c):
                """Close z0(t+1) with the Wh(h1(t)) res-part."""
                mm_half(z0n[c], z0n_chain[c], wh, h1s[c], False, True)
                z0t[c] = z0n[c]
                z0n[c] = None

            # --- per-(chunk, cell) elementwise phases ---
            st = [{}, {}]

            def p_sig(c, cell):
                z = z0t[c] if cell == 0 else z1t[c]
                sio = gates.tile([U, 4 * ch], FP16, name=f"sio{cell}",
                                 tag="sio")
                nc.scalar.activation(sio, z, AF.Sigmoid)
                st[c][f"sio{cell}"] = sio

            def p_dve(c, cell):
                sio = st[c][f"sio{cell}"]
                tgc = (tgc0s if cell == 0 else tgc1s)[c]
                uv = gates.tile([U, 2 * ch], FP16, name="uv", tag="uv")
                # tg = 2*sigmoid(2g) - 1 = tanh(g)
                nc.vector.tensor_scalar(
                    tgc[:, 0:ch], sio[:, 3 * ch:4 * ch],
                    2.0, 1.0, ALU.mult, ALU.subtract)
                # [u|v] = [s_i|s_f] * [tg|c]
                nc.vector.tensor_tensor(
                    out=uv, in0=sio[:, 0:2 * ch], in1=tgc, op=ALU.mult)
                # c_new = u + v  (written into the c half of tgc)
                nc.vector.tensor_tensor(
                    out=tgc[:, ch:2 * ch], in0=uv[:, 0:ch],
                    in1=uv[:, ch:2 * ch], op=ALU.add)

            def p_tanh(c, cell):
                tgc = (tgc0s if cell == 0 else tgc1s)[c]
                tc_t = gates.tile([U, ch], FP16, name="tc", tag="tc")
                nc.scalar.activation(tc_t, tgc[:, ch:2 * ch], AF.Tanh)
                st[c][f"tc{cell}"] = tc_t

            def p_h(c, cell):
                sio = st[c][f"sio{cell}"]
                hstate = (h0s if cell == 0 else h1s)[c]
                nc.vector.tensor_tensor(
                    out=hstate, in0=sio[:, 2 * ch:3 * ch],
                    in1=st[c][f"tc{cell}"], op=ALU.mult)

            def p_z1_rk(c):
                """rk1(h1(t-1)) half of z1(t) - ready early."""
                zt = zpool.tile([U, 4 * ch], FP32, name="z1", tag="z")
                chain = {}
                mm_half(zt, chain, rk1, h1s[c], True, False)
                z1t[c] = zt
                return chain

            def p_z1_k1(c, chain):
                """k1(h0(t)) half of z1(t) - gated on h0. Emitted AFTER all
                dependency-free matmuls of this slot: the PE executes
                matmuls strictly in order, so anything queued behind this
                would otherwise sit idle while h0 is computed."""
                mm_half(z1t[c], chain, k1, h0s[c], False, True)

            def p_wh(c, t):
                """After h1(t): close z0(t+1) with Wh (chain-critical) and
                fold res into acc via wout (off-chain)."""
                if t + 1 < T:
                    finish_z0(c)
                nc.tensor.matmul(accps[c], wout, h1s[c],
                                 start=(t == 0), stop=True,
                                 skip_group_check=(t > 0))

            def p_tail(c, t):
                """acc snapshot -> fp32 history; acc -> fp16 shadow for the
                step-(t+2) kxA matmuls (cast reads the SBUF history copy,
                which is cheaper than a second PSUM read)."""
                b, j = t // tblk, t % tblk
                lo, hi = c * ch, (c + 1) * ch
                nc.vector.tensor_copy(out=ots[b][:, j, lo:hi], in_=accps[c])
                if t + 2 < T:
                    nc.vector.tensor_copy(out=shads[c][t % 2], in_=accps[c])

            # --- prologue: z0(0) for both chunks ---
            for c in range(nch):
                start_z0(c, 0)
                z0_rk0(c, 0)
                z0t[c] = z0n[c]
                z0n[c] = None

            A, B = 0, 1
            for t in range(T):
                b, j = t // tblk, t % tblk
                if j == 0:
                    if b + 2 < nblk and xts[b + 2] is None:
                        alloc_xblock(b + 2)
                    ots[b] = opool.tile([N_OUT, tblk, BL], FP32,
                                        name=f"ob{b}", tag="oblk")
                # slot 1-4: A cell0(t) with B cell1(t-1) trailing
                p_sig(A, 0)
                if t > 0:
                    p_sig(B, 1)
                p_dve(A, 0)
                if t > 0:
                    p_dve(B, 1)
                p_tanh(A, 0)
                if t > 0:
                    p_tanh(B, 1)
                p_h(A, 0)
                if t > 0:
                    p_h(B, 1)
                # slot 5: ready matmuls first (rk1, x, shadow), then the
                # h0-gated ones (k1 before rk0: k1 feeds sig1 of this step)
                chainA = p_z1_rk(A)
                if t + 1 < T:
                    start_z0(A, t + 1)
                p_z1_k1(A, chainA)
                if t + 1 < T:
                    z0_rk0(A, t + 1)
                if t > 0:
                    p_wh(B, t - 1)
                    p_tail(B, t - 1)
                    if j == 0:
                        # B's tail for the last row of block b-1 just ran.
                        nc.sync.dma_start(out=accd[b - 1], in_=ots[b - 1])
                # slot 6-9: A cell1(t) with B cell0(t)
                p_sig(A, 1)
                p_sig(B, 0)
                p_dve(A, 1)
                p_dve(B, 0)
                p_tanh(A, 1)
                p_tanh(B, 0)
                p_h(A, 1)
                p_h(B, 0)
                # slot 10: B's ready matmuls fill the PE while A.h1 and
                # B.h0 are computed; A.Wh (feeds next step's first sigmoid)
                # goes ahead of B's gated k1/rk0.
                chainB = p_z1_rk(B)
                if t + 1 < T:
                    start_z0(B, t + 1)
                p_wh(A, t)
                p_tail(A, t)
                p_z1_k1(B, chainB)
                if t + 1 < T:
                    z0_rk0(B, t + 1)

            # epilogue: B's cell1 of the last step + final DMA
            t = T - 1
            p_sig(B, 1)
            p_dve(B, 1)
            p_tanh(B, 1)
            p_h(B, 1)
            p_wh(B, t)
            p_tail(B, t)
            nc.sync.dma_start(out=accd[nblk - 1], in_=ots[nblk - 1])

    nc.compile()
    meta = dict(T=T, BL=BL, tblk=tblk, nblk=nblk, nch=nch, ch=ch)
    return nc, meta


# Column reorder: reference gate order in z is [i, f, g, o]; PSUM slot
# order is [i, f, o, g] so sigmoid covers slots 0..2 contiguously and g
# sits in the last slot for the tg fix-up.
def _reorder_cols(w):
    u = w.shape[1] // 4
    return np.concatenate(
        [w[:, 0:u], w[:, u:2 * u], w[:, 3 * u:4 * u], w[:, 2 * u:3 * u]], axis=1)


def _prep_z_weight(w):
    """Reorder to [i,f,o,g] and scale the g columns by 2 (sigmoid trick)."""
    w = _reorder_cols(w)
    u = w.shape[1] // 4
    w = w.copy()
    w[:, 3 * u:] *= 2.0
    return w.astype(np.float16)


def prep_weights(W_in, b_in, k0, rk0, bb0, k1, rk1, bb1, W_out, b_out):
    assert np.allclose(b_in, 0) and np.allclose(bb0, 0) and np.allclose(bb1, 0), \
        "nonzero ann/lstm biases not supported by this kernel build"
    k0f = np.asarray(k0, dtype=np.float64)
    Wf = np.asarray(W_in, dtype=np.float64)
    kxX = (Wf[:F_IN] @ k0f).astype(np.float32)          # [64, 512]
    kxA = (Wf[F_IN:] @ k0f).astype(np.float32)          # [32, 512]
    Wh = (np.asarray(W_out, np.float64) @ (Wf[F_IN:] @ k0f)).astype(np.float32)
    return {
        "kxX": _prep_z_weight(kxX),
        "kxA": _prep_z_weight(kxA),
        "wh": _prep_z_weight(Wh),
        "rk0": _prep_z_weight(np.asarray(rk0, np.float32)),
        "k1": _prep_z_weight(np.asarray(k1, np.float32)),
        "rk1": _prep_z_weight(np.asarray(rk1, np.float32)),
        "wout": np.asarray(W_out).astype(np.float16),
    }


def prep_x_core(x_core, tblk):
    """[BL, T, F] fp32 -> [nblk, F, tblk, BL] fp16."""
    BL, T, F = x_core.shape
    nblk = T // tblk
    xt = np.ascontiguousarray(x_core.transpose(1, 2, 0))       # [T, F, BL]
    xt = xt.reshape(nblk, tblk, F, BL).transpose(0, 2, 1, 3)   # [nblk,F,tblk,BL]
    return np.ascontiguousarray(xt).astype(np.float16)


def post_acc_core(acc_hist, b_out):
    """[nblk, 32, tblk, BL] fp32 acc history -> [BL, T, 32] res."""
    nblk, n_out, tblk, BL = acc_hist.shape
    acc = acc_hist.transpose(0, 2, 3, 1).reshape(nblk * tblk, BL, n_out)
    res = np.empty_like(acc)
    res[0] = acc[0]
    np.subtract(acc[1:], acc[:-1], out=res[1:])
    out = res.transpose(1, 0, 2) + b_out.astype(np.float32)
    return np.ascontiguousarray(out.astype(np.float32))


def kernel(inputs, W_in, b_in, k0, rk0, bb0, k1, rk1, bb1, W_out, b_out):
    inputs = np.asarray(inputs, dtype=np.float32)
    W_in, b_in, k0, rk0, bb0, k1, rk1, bb1, W_out, b_out = (
        np.asarray(a, dtype=np.float32)
        for a in (W_in, b_in, k0, rk0, bb0, k1, rk1, bb1, W_out, b_out))
    weights = prep_weights(W_in, b_in, k0, rk0, bb0, k1, rk1, bb1, W_out, b_out)

    nc, meta = build_program()
    in_maps = []
    for r in range(NCORES):
        x_core = inputs[r * BL_FULL:(r + 1) * BL_FULL]
        m = dict(weights)
        m["x"] = prep_x_core(x_core, meta["tblk"])
        in_maps.append(m)

    ret = run_bass_kernel_spmd(nc, in_maps, core_ids=list(range(NCORES)),
                               trace=TRACE)
    if TRACE:
        print("exec_time_ns:", ret.exec_time_ns,
              "mean:", ret.mean_exec_time_ns)
        if ret.instructions_and_trace is not None:
            print("trace:", ret.instructions_and_trace[1])
        kernel.last_results = ret

    out = np.empty((B_FULL, T_FULL, N_OUT), dtype=np.float32)
    for r in range(NCORES):
        out[r * BL_FULL:(r + 1) * BL_FULL] = post_acc_core(
            ret.results[r]["acc_hist"], np.asarray(b_out))
    return out


# revision 30
# speedup vs baseline: 1.1010x; 1.0018x over previous
"""Trainium2 Bass kernel for nn_AccLSTMCell (v2).

Model (per time step t, per batch row):
    a   = concat(x_t, acc) @ W_in + b_in            (linear)
    h0,c0 = LSTMCell(a,  h0, c0; k0, rk0, bb0)
    h1,c1 = LSTMCell(h0, h1, c1; k1, rk1, bb1)
    res = h1 @ W_out + b_out
    acc = acc + res ;  output[t] = res

Key structure (v2):
  * Data parallel: batch 4096 -> 512 rows per NeuronCore (8 cores),
    each core splits its 512 rows into 2 chunks of 256 for pipelining.
  * Layout: features/units on SBUF partitions, batch on free dim.
  * W_in folded into k0 host-side and split into three z0 contributors:
      x-part   Wx  = W_in[:64]  @ k0   (K=64,  dep-free)
      acc-part Wa  = W_in[64:96]@ k0   (K=32,  reads fp16 acc shadow)
      res-part Wh  = W_out @ Wa        (K=128, reads h1 directly)
    z0(t+1) = x(t+1)@Wx + shadow(t-1)@Wa + h0(t)@rk0 + h1(t)@Wh.
    The Wh path keeps the per-step wout->cast sequence OFF the critical
    recurrence chain (the acc shadow cast has a full step of slack).
  * All four gates go through ONE sigmoid per cell: the g columns of
    every z-contributing weight are pre-scaled by 2 host-side, so
    sigmoid(2g) = (tanh(g)+1)/2 and the DVE fixes tg = 2*s_g - 1.
  * fp16 matmuls (fp32 PSUM), fp16 gates/h/c, fp32 acc in PSUM
    (accumulated for free by the wout matmul, start=False).
  * Kernel emits acc history; host recovers res[t] = acc[t]-acc[t-1].
  * Emission interleaves the two chunks at CELL granularity (chunk B
    lags chunk A by one cell) so each engine always has the other
    chunk's ready work behind the current op (hides cross-engine
    semaphore latency; keeps the PE dense so HAM stays at 2.4 GHz).
"""

import os

import numpy as np

import concourse.bass as bass
import concourse.bacc as bacc
import concourse.tile as tile
from concourse import mybir
from concourse.bass_utils import run_bass_kernel_spmd

# Problem constants (hardcoded; harness contract).
B_FULL, T_FULL, F_IN = 4096, 256, 64
N_OUT = 32
ANN = 128          # ann_in dense width
U = 128            # units of both LSTM cells
NCORES = 8
BL_FULL = B_FULL // NCORES   # 512 batch rows per core

FP16 = mybir.dt.float16
FP32 = mybir.dt.float32
AF = mybir.ActivationFunctionType
ALU = mybir.AluOpType

TBLK = int(os.environ.get("LSTM_TBLK", "16"))   # time steps per DMA block
NCH = 2                                          # batch chunks per core
ZBUFS = int(os.environ.get("LSTM_ZBUFS", "3"))
GBUFS = int(os.environ.get("LSTM_GBUFS", "3"))
TRACE = os.environ.get("LSTM_TRACE", "0") == "1"

PSUM_BANK_BYTES = 2048


def build_program(T=T_FULL, BL=BL_FULL, tblk=TBLK, nch=NCH):
    """Build the per-core Bass program. Returns (nc, meta)."""
    assert T % tblk == 0 and BL % nch == 0
    nblk = T // tblk
    ch = BL // nch
    assert nch == 2, "emission interleave assumes two chunks"

    nc = bacc.Bacc("TRN2", name="acclstm2")

    xd = nc.dram_tensor("x", [nblk, F_IN, tblk, BL], FP16, kind="ExternalInput")
    accd = nc.dram_tensor("acc_hist", [nblk, N_OUT, tblk, BL], FP32,
                          kind="ExternalOutput")
    # Weights: column order is PSUM-slot order [i, f, o, g] (host reorders;
    # g columns pre-scaled by 2 for the sigmoid-only gate trick).
    kxXd = nc.dram_tensor("kxX", [F_IN, 4 * U], FP16, kind="ExternalInput")
    kxAd = nc.dram_tensor("kxA", [N_OUT, 4 * U], FP16, kind="ExternalInput")
    whd = nc.dram_tensor("wh", [U, 4 * U], FP16, kind="ExternalInput")
    rk0d = nc.dram_tensor("rk0", [U, 4 * U], FP16, kind="ExternalInput")
    k1d = nc.dram_tensor("k1", [U, 4 * U], FP16, kind="ExternalInput")
    rk1d = nc.dram_tensor("rk1", [U, 4 * U], FP16, kind="ExternalInput")
    woutd = nc.dram_tensor("wout", [U, N_OUT], FP16, kind="ExternalInput")

    slots_per_bank = max(1, PSUM_BANK_BYTES // (ch * 4))

    with tile.TileContext(nc) as tc:
        with (
            tc.tile_pool(name="wpool", bufs=1) as wpool,
            tc.tile_pool(name="state", bufs=1) as state,
            tc.tile_pool(name="xpool", bufs=2) as xpool,
            tc.tile_pool(name="opool", bufs=2) as opool,
            tc.tile_pool(name="gates", bufs=GBUFS) as gates,
            tc.tile_pool(name="zpool", bufs=ZBUFS, space="PSUM") as zpool,
            tc.tile_pool(name="apool", bufs=1, space="PSUM") as apool,
        ):
            # --- weights -> SBUF ---
            def wload(nm, dram, shape):
                t_ = wpool.tile(shape, FP16, name=nm, tag=nm)
                nc.sync.dma_start(out=t_, in_=dram[:, :])
                return t_
            kxX = wload("kxX", kxXd, [F_IN, 4 * U])
            kxA = wload("kxA", kxAd, [N_OUT, 4 * U])
            wh = wload("wh", whd, [U, 4 * U])
            rk0 = wload("rk0", rk0d, [U, 4 * U])
            k1 = wload("k1", k1d, [U, 4 * U])
            rk1 = wload("rk1", rk1d, [U, 4 * U])
            wout = wload("wout", woutd, [U, N_OUT])

            # --- persistent per-chunk state ---
            h0s, h1s, tgc0s, tgc1s, shads = [], [], [], [], []
            for c in range(nch):
                h0 = state.tile([U, ch], FP16, name=f"h0_{c}", tag=f"h0_{c}")
                h1 = state.tile([U, ch], FP16, name=f"h1_{c}", tag=f"h1_{c}")
                # [tg | c] combined tile per cell: tg scratch, c persistent.
                tgc0 = state.tile([U, 2 * ch], FP16, name=f"tgc0_{c}",
                                  tag=f"tgc0_{c}")
                tgc1 = state.tile([U, 2 * ch], FP16, name=f"tgc1_{c}",
                                  tag=f"tgc1_{c}")
                sh = [state.tile([N_OUT, ch], FP16, name=f"sh{k}_{c}",
                                 tag=f"sh{k}_{c}") for k in range(2)]
                for tl in (h0, h1, tgc0, tgc1, sh[0], sh[1]):
                    nc.vector.memset(tl, 0.0)
                h0s.append(h0); h1s.append(h1)
                tgc0s.append(tgc0); tgc1s.append(tgc1); shads.append(sh)

            # fp32 acc accumulators live in PSUM, one bank per chunk,
            # updated by the wout matmul itself (start=False accumulate).
            accps = [apool.tile([N_OUT, ch], FP32, name=f"accps{c}",
                                tag=f"accps{c}") for c in range(nch)]

            # --- x input blocks / output history blocks ---
            xts = [None] * nblk
            ots = [None] * nblk

            def alloc_xblock(b):
                xts[b] = xpool.tile([F_IN, tblk, BL], FP16,
                                    name=f"xb{b}", tag="xblk")
                nc.sync.dma_start(out=xts[b], in_=xd[b])

            alloc_xblock(0)
            alloc_xblock(1)

            def mm_half(z, chain, w, rhs, is_first, is_last):
                """Emit the 4 gate-slot matmuls of one z contributor.
                Within each 2KB PSUM bank the first executed matmul must
                carry start=True and the last stop=True; `chain` pins the
                execution order inside each bank with same-engine deps."""
                for bank0 in range(0, 4, slots_per_bank):
                    bslots = list(range(bank0, min(bank0 + slots_per_bank, 4)))
                    bk = bank0 // slots_per_bank
                    for i, s in enumerate(bslots):
                        mm = nc.tensor.matmul(
                            z[:, s * ch:(s + 1) * ch],
                            w[:, s * U:(s + 1) * U],
                            rhs,
                            start=(is_first and i == 0),
                            stop=(is_last and i == len(bslots) - 1),
                        )
                        prev = chain.get(bk)
                        if prev is not None:
                            tile.add_dep_helper(
                                mm.ins, prev.ins, sync=False,
                                reason="psum bank group order")
                        chain[bk] = mm

            # Per-chunk in-flight z tiles / chains.
            z0t = [None] * nch    # z0 tile consumed by sig0 at step t
            z1t = [None] * nch
            z0n = [None] * nch    # z0 tile being assembled for step t+1
            z0n_chain = [None] * nch

            def start_z0(c, t):
                """Open z0 for step t with the dependency-free parts:
                x(t) + shadow[acc(t-2)]. These execute immediately, keeping
                the PE busy (and HAM warm) while h0/h1 are still being
                computed. rk0(h0(t-1)) and Wh(h1(t-1)) are appended later:
                the PE runs matmuls strictly in order, so gated matmuls
                must be emitted AFTER every ready one."""
                zt = zpool.tile([U, 4 * ch], FP32, name="z0", tag="z")
                chain = {}
                b, j = t // tblk, t % tblk
                lo, hi = c * ch, (c + 1) * ch
                mm_half(zt, chain, kxX, xts[b][:, j, lo:hi], True, False)
                mm_half(zt, chain, kxA, shads[c][(t - 2) % 2], False, False)
                z0n[c] = zt
                z0n_chain[c] = chain

            def z0_rk0(c, t):
                """rk0(h0(t-1)) part of z0(t) - gated on h0."""
                mm_half(z0n[c], z0n_chain[c], rk0, h0s[c], False, t == 0)

            def finish_z0(c):
                """Close z0(t+1) with the Wh(h1(t)) res-part."""
                mm_half(z0n[c], z0n_chain[c], wh, h1s[c], False, True)
                z0t[c] = z0n[c]
                z0n[c] = None

            # --- per-(chunk, cell) elementwise phases ---
            st = [{}, {}]

            def p_sig(c, cell):
                z = z0t[c] if cell == 0 else z1t[c]
                sio = gates.tile([U, 4 * ch], FP16, name=f"sio{cell}",
                                 tag="sio")
                nc.scalar.activation(sio, z, AF.Sigmoid)
                st[c][f"sio{cell}"] = sio

            def p_dve_a(c, cell):
                """tg fix-up + [u|v] product."""
                sio = st[c][f"sio{cell}"]
                tgc = (tgc0s if cell == 0 else tgc1s)[c]
                uv = gates.tile([U, 2 * ch], FP16, name="uv", tag="uv")
                # tg = 2*sigmoid(2g) - 1 = tanh(g)
                nc.vector.tensor_scalar(
                    tgc[:, 0:ch], sio[:, 3 * ch:4 * ch],
                    2.0, 1.0, ALU.mult, ALU.subtract)
                # [u|v] = [s_i|s_f] * [tg|c]
                nc.vector.tensor_tensor(
                    out=uv, in0=sio[:, 0:2 * ch], in1=tgc, op=ALU.mult)
                st[c][f"uv{cell}"] = uv

            def p_dve_b(c, cell):
                """c_new = u + v (written into the c half of tgc)."""
                uv = st[c][f"uv{cell}"]
                tgc = (tgc0s if cell == 0 else tgc1s)[c]
                nc.vector.tensor_tensor(
                    out=tgc[:, ch:2 * ch], in0=uv[:, 0:ch],
                    in1=uv[:, ch:2 * ch], op=ALU.add)

            def p_dve(c, cell):
                p_dve_a(c, cell)
                p_dve_b(c, cell)

            def p_tanh(c, cell):
                tgc = (tgc0s if cell == 0 else tgc1s)[c]
                tc_t = gates.tile([U, ch], FP16, name="tc", tag="tc")
                nc.scalar.activation(tc_t, tgc[:, ch:2 * ch], AF.Tanh)
                st[c][f"tc{cell}"] = tc_t

            def p_h(c, cell):
                sio = st[c][f"sio{cell}"]
                hstate = (h0s if cell == 0 else h1s)[c]
                nc.vector.tensor_tensor(
                    out=hstate, in0=sio[:, 2 * ch:3 * ch],
                    in1=st[c][f"tc{cell}"], op=ALU.mult)

            def p_z1_rk(c):
                """rk1(h1(t-1)) half of z1(t) - ready early."""
                zt = zpool.tile([U, 4 * ch], FP32, name="z1", tag="z")
                chain = {}
                mm_half(zt, chain, rk1, h1s[c], True, False)
                z1t[c] = zt
                return chain

            def p_z1_k1(c, chain):
                """k1(h0(t)) half of z1(t) - gated on h0. Emitted AFTER all
                dependency-free matmuls of this slot: the PE executes
                matmuls strictly in order, so anything queued behind this
                would otherwise sit idle while h0 is computed."""
                mm_half(z1t[c], chain, k1, h0s[c], False, True)

            def p_wh(c, t):
                """After h1(t): close z0(t+1) with Wh (chain-critical) and
                fold res into acc via wout (off-chain)."""
                if t + 1 < T:
                    finish_z0(c)
                nc.tensor.matmul(accps[c], wout, h1s[c],
                                 start=(t == 0), stop=True,
                                 skip_group_check=(t > 0))

            def p_tail(c, t):
                """acc snapshot -> fp32 history; acc -> fp16 shadow for the
                step-(t+2) kxA matmuls (cast reads the SBUF history copy,
                which is cheaper than a second PSUM read)."""
                b, j = t // tblk, t % tblk
                lo, hi = c * ch, (c + 1) * ch
                nc.vector.tensor_copy(out=ots[b][:, j, lo:hi], in_=accps[c])
                if t + 2 < T:
                    nc.vector.tensor_copy(out=shads[c][t % 2], in_=accps[c])

            # --- prologue: z0(0) for both chunks ---
            for c in range(nch):
                start_z0(c, 0)
                z0_rk0(c, 0)
                z0t[c] = z0n[c]
                z0n[c] = None

            A, B = 0, 1
            for t in range(T):
                b, j = t // tblk, t % tblk
                if j == 0:
                    if b + 2 < nblk and xts[b + 2] is None:
                        alloc_xblock(b + 2)
                    ots[b] = opool.tile([N_OUT, tblk, BL], FP32,
                                        name=f"ob{b}", tag="oblk")
                # slot 1-4: A cell0(t) with B cell1(t-1) trailing.
                # B's c-add is emitted AFTER A's h-mul on the DVE: h(A,0)
                # becomes ready before B's trio completes, and the strict
                # DVE FIFO would otherwise hold it behind B's c-add.
                p_sig(A, 0)
                if t > 0:
                    p_sig(B, 1)
                p_dve(A, 0)
                p_tanh(A, 0)
                if t > 0:
                    p_dve_a(B, 1)
                p_h(A, 0)
                if t > 0:
                    p_dve_b(B, 1)
                    p_tanh(B, 1)
                    p_h(B, 1)
                # slot 5: ready matmuls first (rk1, x, shadow), then the
                # h0-gated ones (k1 before rk0: k1 feeds sig1 of this step)
                chainA = p_z1_rk(A)
                if t + 1 < T:
                    start_z0(A, t + 1)
                p_z1_k1(A, chainA)
                if t + 1 < T:
                    z0_rk0(A, t + 1)
                if t > 0:
                    p_wh(B, t - 1)
                    p_tail(B, t - 1)
                    if j == 0:
                        # B's tail for the last row of block b-1 just ran.
                        nc.sync.dma_start(out=accd[b - 1], in_=ots[b - 1])
                # slot 6-9: A cell1(t) with B cell0(t), same DVE split
                p_sig(A, 1)
                p_sig(B, 0)
                p_dve(A, 1)
                p_tanh(A, 1)
                p_dve_a(B, 0)
                p_h(A, 1)
                p_dve_b(B, 0)
                p_tanh(B, 0)
                p_h(B, 0)
                # slot 10: B's ready matmuls fill the PE while A.h1 and
                # B.h0 are computed; A.Wh (feeds next step's first sigmoid)
                # goes ahead of B's gated k1/rk0.
                chainB = p_z1_rk(B)
                if t + 1 < T:
                    start_z0(B, t + 1)
                p_wh(A, t)
                p_tail(A, t)
                p_z1_k1(B, chainB)
                if t + 1 < T:
                    z0_rk0(B, t + 1)

            # epilogue: B's cell1 of the last step + final DMA
            t = T - 1
            p_sig(B, 1)
            p_dve(B, 1)
            p_tanh(B, 1)
            p_h(B, 1)
            p_wh(B, t)
            p_tail(B, t)
            nc.sync.dma_start(out=accd[nblk - 1], in_=ots[nblk - 1])

    nc.compile()
    meta = dict(T=T, BL=BL, tblk=tblk, nblk=nblk, nch=nch, ch=ch)
    return nc, meta


# Column reorder: reference gate order in z is [i, f, g, o]; PSUM slot
# order is [i, f, o, g] so sigmoid covers slots 0..2 contiguously and g
# sits in the last slot for the tg fix-up.
def _reorder_cols(w):
    u = w.shape[1] // 4
    return np.concatenate(
        [w[:, 0:u], w[:, u:2 * u], w[:, 3 * u:4 * u], w[:, 2 * u:3 * u]], axis=1)


def _prep_z_weight(w):
    """Reorder to [i,f,o,g] and scale the g columns by 2 (sigmoid trick)."""
    w = _reorder_cols(w)
    u = w.shape[1] // 4
    w = w.copy()
    w[:, 3 * u:] *= 2.0
    return w.astype(np.float16)


def prep_weights(W_in, b_in, k0, rk0, bb0, k1, rk1, bb1, W_out, b_out):
    assert np.allclose(b_in, 0) and np.allclose(bb0, 0) and np.allclose(bb1, 0), \
        "nonzero ann/lstm biases not supported by this kernel build"
    k0f = np.asarray(k0, dtype=np.float64)
    Wf = np.asarray(W_in, dtype=np.float64)
    kxX = (Wf[:F_IN] @ k0f).astype(np.float32)          # [64, 512]
    kxA = (Wf[F_IN:] @ k0f).astype(np.float32)          # [32, 512]
    Wh = (np.asarray(W_out, np.float64) @ (Wf[F_IN:] @ k0f)).astype(np.float32)
    return {
        "kxX": _prep_z_weight(kxX),
        "kxA": _prep_z_weight(kxA),
        "wh": _prep_z_weight(Wh),
        "rk0": _prep_z_weight(np.asarray(rk0, np.float32)),
        "k1": _prep_z_weight(np.asarray(k1, np.float32)),
        "rk1": _prep_z_weight(np.asarray(rk1, np.float32)),
        "wout": np.asarray(W_out).astype(np.float16),
    }


def prep_x_core(x_core, tblk):
    """[BL, T, F] fp32 -> [nblk, F, tblk, BL] fp16."""
    BL, T, F = x_core.shape
    nblk = T // tblk
    xt = np.ascontiguousarray(x_core.transpose(1, 2, 0))       # [T, F, BL]
    xt = xt.reshape(nblk, tblk, F, BL).transpose(0, 2, 1, 3)   # [nblk,F,tblk,BL]
    return np.ascontiguousarray(xt).astype(np.float16)


def post_acc_core(acc_hist, b_out):
    """[nblk, 32, tblk, BL] fp32 acc history -> [BL, T, 32] res."""
    nblk, n_out, tblk, BL = acc_hist.shape
    acc = acc_hist.transpose(0, 2, 3, 1).reshape(nblk * tblk, BL, n_out)
    res = np.empty_like(acc)
    res[0] = acc[0]
    np.subtract(acc[1:], acc[:-1], out=res[1:])
    out = res.transpose(1, 0, 2) + b_out.astype(np.float32)
    return np.ascontiguousarray(out.astype(np.float32))


def kernel(inputs, W_in, b_in, k0, rk0, bb0, k1, rk1, bb1, W_out, b_out):
    inputs = np.asarray(inputs, dtype=np.float32)
    W_in, b_in, k0, rk0, bb0, k1, rk1, bb1, W_out, b_out = (
        np.asarray(a, dtype=np.float32)
        for a in (W_in, b_in, k0, rk0, bb0, k1, rk1, bb1, W_out, b_out))
    weights = prep_weights(W_in, b_in, k0, rk0, bb0, k1, rk1, bb1, W_out, b_out)

    nc, meta = build_program()
    in_maps = []
    for r in range(NCORES):
        x_core = inputs[r * BL_FULL:(r + 1) * BL_FULL]
        m = dict(weights)
        m["x"] = prep_x_core(x_core, meta["tblk"])
        in_maps.append(m)

    ret = run_bass_kernel_spmd(nc, in_maps, core_ids=list(range(NCORES)),
                               trace=TRACE)
    if TRACE:
        print("exec_time_ns:", ret.exec_time_ns,
              "mean:", ret.mean_exec_time_ns)
        if ret.instructions_and_trace is not None:
            print("trace:", ret.instructions_and_trace[1])
        kernel.last_results = ret

    out = np.empty((B_FULL, T_FULL, N_OUT), dtype=np.float32)
    for r in range(NCORES):
        out[r * BL_FULL:(r + 1) * BL_FULL] = post_acc_core(
            ret.results[r]["acc_hist"], np.asarray(b_out))
    return out
